# revision 9
# baseline (speedup 1.0000x reference)
"""CausalGraphBuilder Trainium2 kernel.

Full inputs -> shard batch (128) over 8 NeuronCores (16 each) -> Bass kernel
(encoder MLP + LayerNorm + N^2 pair-MLP edge/strength heads) -> gather.

Self-contained: hardcodes B,S,D,N = 128,1024,512,64 and the sharding.
"""

import numpy as np

B, S, D, N = 128, 1024, 512, 64
DM = D // 4          # 128 node-feature dim after encoder
NCORES = 8
BSH = B // NCORES    # 16 batch per core
TOK = BSH * N        # 1024 tokens per core

# jnp.linspace(0, S-1, N).astype(int32) as evaluated by the reference in this
# environment (device cast rounds); recomputed at runtime when jax is available.
_FALLBACK_IDX = [0, 16, 32, 49, 65, 81, 97, 114, 130, 146, 162, 179, 195, 211,
                 227, 244, 260, 276, 292, 309, 325, 341, 357, 373, 390, 406,
                 422, 438, 455, 471, 487, 503, 520, 536, 552, 568, 585, 601,
                 617, 633, 650, 666, 682, 698, 714, 731, 747, 763, 779, 796,
                 812, 828, 844, 861, 877, 893, 909, 926, 942, 958, 974, 991,
                 1007, 1023]


def _node_indices():
    try:
        import jax.numpy as jnp
        idx = np.asarray(jnp.linspace(0.0, float(S - 1), N).astype(jnp.int32))
        if idx.shape == (N,):
            return idx.astype(np.int64)
    except Exception:
        pass
    return np.array(_FALLBACK_IDX, dtype=np.int64)


# ---------------------------------------------------------------------------
# device program
# ---------------------------------------------------------------------------

_PROGRAM_CACHE = {}
LAST_RESULTS = None  # BassKernelResults of the most recent run (for test.py)

# engine assignment knobs (tuned against the profile)
PE_E_TILES = (0, 2, 4, 6)     # E-groups built on PE (identity-matmul 2-pass)
PE_S_TILES = (0, 1, 2, 3)     # S-groups built on PE
EVAC_DVE_E = (4, 6)           # PE-built E tiles evacuated by DVE instead of ACT
EVAC_DVE_S = (2, 3)
GP_RELU = True                # DVE-built tiles: relu on GPSIMD
E2RELU_DVE = (1, 3)           # e2 cp-chunks (0..3) whose relu-evac runs on DVE


def _patched_tile_context(nc):
    """TileContext whose tail drain never carries >1 sync wait (this walrus
    build rejects multi-wait CTRL instructions)."""
    import concourse.mybir as mybir
    import concourse.tile as tile
    from concourse.vector_clock import ScopedClock

    class TileContextP(tile.TileContext):
        def _drain_and_barrier(self, tick_clock, wait_clock):
            drain_inst = self.nc.sync.drain()
            wait_clock.add_sem_waits(
                drain_inst.ins, ScopedClock({None: tick_clock.global_clock})
            )
            si = drain_inst.ins.sync_info
            if si is not None and si.on_wait is not None and len(si.on_wait) > 1:
                waits = list(si.on_wait)
                si.on_wait = waits[:1]
                for w in waits[1:]:
                    extra = self.nc.sync.drain()
                    extra.ins.sync_info = mybir.SyncInfo(on_wait=[w], on_update=[])
            self.nc.all_engine_barrier()
            popped = self.nc._tile_sem_poison_stack.pop()
            assert popped is self._sem_poison
            self.nc.clear_and_free_semaphores(list(self.sems.allocated().values()))
            self.nc.all_engine_barrier()

    return TileContextP(nc)


def _split_multiwait(nc):
    """This walrus build accepts at most ONE sync wait per instruction; hoist
    extra waits into single-wait NoOps on the same engine just before."""
    import concourse.mybir as mybir

    n_split = 0
    for f in nc.m.functions:
        for bb in f.blocks:
            insts = list(bb.instructions)
            out = []
            for ins in insts:
                si = ins.sync_info
                if si is not None and si.on_wait is not None and len(si.on_wait) > 1:
                    waits = list(si.on_wait)
                    for w in waits[:-1]:
                        nop = mybir.InstNoOp(
                            name=f"{ins.name}-w{n_split}",
                            engine=ins.engine,
                            bass_nofuse=True,
                            sync_info=mybir.SyncInfo(on_wait=[w], on_update=[]),
                        )
                        out.append(nop)
                        n_split += 1
                    si.on_wait = waits[-1:]
                out.append(ins)
            if n_split:
                bb.instructions = out
    return n_split


def _build_program():
    import concourse.bass as bass
    import concourse.mybir as mybir
    from concourse.masks import make_identity

    f16 = mybir.dt.float16
    f32 = mybir.dt.float32
    AF = mybir.ActivationFunctionType
    OP = mybir.AluOpType

    nc = bass.Bass()

    # ---- DRAM I/O ----
    xt_d = nc.dram_tensor("xt", [D, TOK], f16, kind="ExternalInput")
    w1_d = nc.dram_tensor("w1", [D, 2 * DM], f16, kind="ExternalInput")
    w2_d = nc.dram_tensor("w2", [2 * DM, DM], f16, kind="ExternalInput")
    b1_d = nc.dram_tensor("b1_2", [128, 2], f32, kind="ExternalInput")
    b2_d = nc.dram_tensor("b2c", [128, 1], f32, kind="ExternalInput")
    gp_d = nc.dram_tensor("gammap", [128, 1], f32, kind="ExternalInput")
    bp_d = nc.dram_tensor("betap", [128, 1], f32, kind="ExternalInput")
    we1a_d = nc.dram_tensor("we1a", [128, 64], f16, kind="ExternalInput")
    we1b_d = nc.dram_tensor("we1b", [128, 64], f16, kind="ExternalInput")
    ws1a_d = nc.dram_tensor("ws1a", [128, 32], f16, kind="ExternalInput")
    ws1b_d = nc.dram_tensor("ws1b", [128, 32], f16, kind="ExternalInput")
    be1_d = nc.dram_tensor("be1_2", [128, 1], f32, kind="ExternalInput")
    bs1_d = nc.dram_tensor("bs1_4", [128, 1], f32, kind="ExternalInput")
    be2_d = nc.dram_tensor("be2_4", [128, 1], f32, kind="ExternalInput")
    bd2we2_d = nc.dram_tensor("bd2we2", [128, 64], f16, kind="ExternalInput")
    bd4we3_d = nc.dram_tensor("bd4we3", [128, 4], f16, kind="ExternalInput")
    bd4ws2_d = nc.dram_tensor("bd4ws2", [128, 4], f16, kind="ExternalInput")
    be3_d = nc.dram_tensor("be3b", [128, 1], f32, kind="ExternalInput")
    bs2_d = nc.dram_tensor("bs2b", [128, 1], f32, kind="ExternalInput")

    nf_d = nc.dram_tensor("nf_raw", [TOK, DM], f32, kind="ExternalOutput")
    adj_d = nc.dram_tensor("adj", [BSH, N * N], f32, kind="ExternalOutput")
    str_d = nc.dram_tensor("strg", [BSH, N * N], f32, kind="ExternalOutput")

    with _patched_tile_context(nc) as tc:
        from contextlib import ExitStack

        with ExitStack() as ctx:
            consts = ctx.enter_context(tc.tile_pool(name="consts", bufs=1))
            persist = ctx.enter_context(tc.tile_pool(name="persist", bufs=1))

            # ---- load constants ----
            def load(pool, name, dram, shape, dtype):
                t = pool.tile(shape, dtype, tag=name, name=name)
                nc.sync.dma_start(out=t[:], in_=dram[:])
                return t

            xt_sb = [load(consts, f"xt{c}", xt_d[c * 128:(c + 1) * 128, :],
                          [128, TOK], f16) for c in range(4)]
            w1_sb = [load(consts, f"w1_{c}", w1_d[c * 128:(c + 1) * 128, :],
                          [128, 2 * DM], f16) for c in range(4)]
            w2_sb = [load(consts, f"w2_{c}", w2_d[c * 128:(c + 1) * 128, :],
                          [128, DM], f16) for c in range(2)]
            b1_sb = load(consts, "b1", b1_d, [128, 2], f32)
            b2_sb = load(consts, "b2", b2_d, [128, 1], f32)
            gp_sb = load(consts, "gp", gp_d, [128, 1], f32)
            bp_sb = load(consts, "bp", bp_d, [128, 1], f32)
            we1a_sb = load(consts, "we1a", we1a_d, [128, 64], f16)
            we1b_sb = load(consts, "we1b", we1b_d, [128, 64], f16)
            ws1a_sb = load(consts, "ws1a", ws1a_d, [128, 32], f16)
            ws1b_sb = load(consts, "ws1b", ws1b_d, [128, 32], f16)
            be1_sb = load(consts, "be1", be1_d, [128, 1], f32)
            bs1_sb = load(consts, "bs1", bs1_d, [128, 1], f32)
            be2_sb = load(consts, "be2", be2_d, [128, 1], f32)
            bd2we2_sb = load(consts, "bd2we2", bd2we2_d, [128, 64], f16)
            bd4we3_sb = load(consts, "bd4we3", bd4we3_d, [128, 4], f16)
            bd4ws2_sb = load(consts, "bd4ws2", bd4ws2_d, [128, 4], f16)
            be3_sb = load(consts, "be3", be3_d, [128, 1], f32)
            bs2_sb = load(consts, "bs2", bs2_d, [128, 1], f32)

            eps_sb = consts.tile([128, 1], f32, tag="eps")
            nc.vector.memset(eps_sb[:], 1e-5)

            idf32 = consts.tile([128, 128], f32, tag="idf32")
            make_identity(nc, idf32[:])
            idf16 = consts.tile([128, 128], f16, tag="idf16")
            make_identity(nc, idf16[:])

            h1t = [persist.tile([128, TOK], f16, tag=f"h1t{m}", name=f"h1t{m}") for m in range(2)]
            ht = persist.tile([128, TOK], f32, tag="ht")
            nfraw = persist.tile([128, 8, 128], f32, tag="nfraw")
            nft = persist.tile([128, TOK], f16, tag="nft")
            ab = [persist.tile([128, 128], f16, tag=f"ab{g}", name=f"ab{g}") for g in range(8)]
            apbp = [persist.tile([128, 128], f16, tag=f"apbp{gr}", name=f"apbp{gr}") for gr in range(4)]
            e2r = [persist.tile([128, N * N], f16, tag=f"e2r{u}", name=f"e2r{u}") for u in range(4)]
            s1r = [persist.tile([128, N * N], f16, tag=f"s1r{gr}", name=f"s1r{gr}") for gr in range(4)]

            # ================= encoder =================
            with tc.tile_pool(name="ps_enc", bufs=2, space="PSUM") as ps_enc:
                for m in range(2):
                    for n2 in range(2):
                        ps = ps_enc.tile([128, 512], f32, tag="enc", name="ps_enc_t")
                        for c in range(4):
                            nc.tensor.matmul(
                                ps[:],
                                lhsT=w1_sb[c][:, m * 128:(m + 1) * 128],
                                rhs=xt_sb[c][:, n2 * 512:(n2 + 1) * 512],
                                start=(c == 0), stop=(c == 3),
                            )
                        nc.scalar.activation(
                            out=h1t[m][:, n2 * 512:(n2 + 1) * 512], in_=ps[:],
                            func=AF.Relu, bias=b1_sb[:, m:m + 1], scale=1.0,
                        )
                for n2 in range(2):
                    ps = ps_enc.tile([128, 512], f32, tag="enc", name="ps_enc_t")
                    for c2 in range(2):
                        nc.tensor.matmul(
                            ps[:], lhsT=w2_sb[c2][:],
                            rhs=h1t[c2][:, n2 * 512:(n2 + 1) * 512],
                            start=(c2 == 0), stop=(c2 == 1),
                        )
                    nc.scalar.activation(
                        out=ht[:, n2 * 512:(n2 + 1) * 512], in_=ps[:],
                        func=AF.Identity, bias=b2_sb[:, 0:1], scale=1.0,
                    )

            # ================= layernorm (token-major) =================
            with tc.tile_pool(name="ps_tr", bufs=3, space="PSUM") as ps_tr, \
                 tc.tile_pool(name="ln_tmp", bufs=4) as ln_tmp:
                for t in range(8):
                    pst = ps_tr.tile([128, 128], f32, tag="htok", name="pst")
                    nc.tensor.transpose(pst[:], ht[:, t * 128:(t + 1) * 128],
                                        idf32[:])
                    st6 = ln_tmp.tile([128, 6], f32, tag="st6", name="st6")
                    nc.vector.bn_stats(out=st6[:], in_=pst[:])
                    mv = ln_tmp.tile([128, 2], f32, tag="mv", name="mv")
                    nc.vector.bn_aggr(out=mv[:], in_=st6[:])
                    rstd = ln_tmp.tile([128, 1], f32, tag="rstd", name="rstd")
                    nc.scalar.activation(out=rstd[:], in_=mv[:, 1:2],
                                         func=AF.Sqrt, bias=eps_sb[:, 0:1],
                                         scale=1.0)
                    nc.vector.reciprocal(out=rstd[:], in_=rstd[:])
                    nc.vector.tensor_scalar(
                        out=nfraw[:, t, :], in0=pst[:],
                        scalar1=mv[:, 0:1], scalar2=rstd[:, 0:1],
                        op0=OP.subtract, op1=OP.mult,
                    )
                    nc.sync.dma_start(out=nf_d[t * 128:(t + 1) * 128, :],
                                      in_=nfraw[:, t, :])
                # nfT (feature-major) with gamma/beta applied per-partition
                for t in range(8):
                    psn = ps_tr.tile([128, 128], f32, tag="nft_ps", name="psn")
                    nc.tensor.transpose(psn[:], nfraw[:, t, :], idf32[:])
                    nc.vector.tensor_scalar(
                        out=nft[:, t * 128:(t + 1) * 128], in0=psn[:],
                        scalar1=gp_sb[:, 0:1], scalar2=bp_sb[:, 0:1],
                        op0=OP.mult, op1=OP.add,
                    )

            # ================= pair projections =================
            with tc.tile_pool(name="ps_proj", bufs=3, space="PSUM") as ps_proj:
                for g in range(8):           # edge groups: b = 2g, 2g+1
                    ps = ps_proj.tile([128, 128], f32, tag="proj_e", name="ps_proj_e")
                    for bb in range(2):
                        b = 2 * g + bb
                        cols = nft[:, b * N:(b + 1) * N]
                        nc.tensor.matmul(
                            ps[bb * 64:(bb + 1) * 64, 0:64], lhsT=we1a_sb[:],
                            rhs=cols, start=True, stop=True,
                            tile_position=(0, 64 * bb),
                        )
                        nc.tensor.matmul(
                            ps[bb * 64:(bb + 1) * 64, 64:128], lhsT=we1b_sb[:],
                            rhs=cols, start=True, stop=True,
                            tile_position=(0, 64 * bb),
                        )
                    nc.scalar.activation(out=ab[g][:], in_=ps[:], func=AF.Copy)
                for gr in range(4):          # strength groups: b = 4gr..4gr+3
                    ps = ps_proj.tile([128, 128], f32, tag="proj_s", name="ps_proj_s")
                    for bb in range(4):
                        b = 4 * gr + bb
                        cols = nft[:, b * N:(b + 1) * N]
                        nc.tensor.matmul(
                            ps[bb * 32:(bb + 1) * 32, 0:64], lhsT=ws1a_sb[:],
                            rhs=cols, start=True, stop=True,
                            tile_position=(0, 32 * bb),
                        )
                        nc.tensor.matmul(
                            ps[bb * 32:(bb + 1) * 32, 64:128], lhsT=ws1b_sb[:],
                            rhs=cols, start=True, stop=True,
                            tile_position=(0, 32 * bb),
                        )
                    nc.scalar.activation(out=apbp[gr][:], in_=ps[:], func=AF.Copy)

            # ================= N^2 builds + edge layer2 =================
            with tc.tile_pool(name="e1pool", bufs=3) as e1pool, \
                 tc.tile_pool(name="ps_bld", bufs=2, space="PSUM") as ps_bld, \
                 tc.tile_pool(name="ps_e2", bufs=2, space="PSUM") as ps_e2:
                e1_tiles = {}

                def build_pe(out_flat, src, bias, evac_dve):
                    """2-pass identity-matmul broadcast build + relu evac.

                    src: [128, 128] sbuf tile, A in cols 0:64, B in cols 64:128.
                    out_flat: [128, 4096] fp16 destination (relu'd)."""
                    for q in range(4):           # 1024-pair chunks (16 i vals)
                        ps = ps_bld.tile([128, 1024], f32, tag="bld",
                                         name="ps_bld_t")
                        for hf in range(2):      # 512-col matmuls (8 i vals)
                            i0 = q * 16 + hf * 8
                            a_chunk = src[:, i0:i0 + 8][:, :, None] \
                                .broadcast_to((128, 8, 64))
                            b_chunk = src[:, None, 64:128] \
                                .broadcast_to((128, 8, 64))
                            o = ps[:, hf * 512:(hf + 1) * 512]
                            nc.tensor.matmul(o, lhsT=idf16[:], rhs=a_chunk,
                                             start=True, stop=False)
                            nc.tensor.matmul(o, lhsT=idf16[:], rhs=b_chunk,
                                             start=False, stop=True)
                        dst = out_flat[:, q * 1024:(q + 1) * 1024]
                        if evac_dve:
                            nc.vector.tensor_scalar(
                                out=dst, in0=ps[:], scalar1=bias,
                                scalar2=0.0, op0=OP.add, op1=OP.max)
                        else:
                            nc.scalar.activation(out=dst, in_=ps[:],
                                                 func=AF.Relu, bias=bias,
                                                 scale=1.0)

                def build_dve(out3, src, bias):
                    a_view = src[:, 0:64][:, :, None].broadcast_to((128, N, N))
                    b_view = src[:, None, 64:128].broadcast_to((128, N, N))
                    nc.vector.scalar_tensor_tensor(
                        out=out3, in0=b_view, scalar=bias,
                        in1=a_view, op0=OP.add, op1=OP.add,
                    )
                    relu_eng = nc.gpsimd if GP_RELU else nc.vector
                    relu_eng.tensor_scalar_max(out=out3, in0=out3, scalar1=0.0)

                def build_edge(g):
                    e1 = e1pool.tile([128, N, N], f16, tag="e1", name=f"e1_{g}")
                    if g in PE_E_TILES:
                        build_pe(e1[:].rearrange("p i j -> p (i j)"), ab[g][:],
                                 be1_sb[:, 0:1], g in EVAC_DVE_E)
                    else:
                        build_dve(e1[:], ab[g][:], be1_sb[:, 0:1])
                    e1_tiles[g] = e1

                def build_strength(gr):
                    s1 = s1r[gr]
                    s3 = s1[:].rearrange("p (i j) -> p i j", i=N)
                    if gr in PE_S_TILES:
                        build_pe(s1[:], apbp[gr][:], bs1_sb[:, 0:1],
                                 gr in EVAC_DVE_S)
                    else:
                        build_dve(s3, apbp[gr][:], bs1_sb[:, 0:1])

                for u in range(4):
                    build_edge(2 * u)
                    build_edge(2 * u + 1)
                    build_strength(u)
                    e1a = e1_tiles[2 * u][:].rearrange("p i j -> p (i j)")
                    e1b = e1_tiles[2 * u + 1][:].rearrange("p i j -> p (i j)")
                    for cp in range(4):
                        ps = ps_e2.tile([128, 1024], f32, tag="e2", name="ps_e2_t")
                        for hf in range(2):
                            c0 = cp * 1024 + hf * 512
                            nc.tensor.matmul(
                                ps[0:64, hf * 512:(hf + 1) * 512],
                                lhsT=bd2we2_sb[:], rhs=e1a[:, c0:c0 + 512],
                                start=True, stop=True, tile_position=(0, 0),
                            )
                            nc.tensor.matmul(
                                ps[64:128, hf * 512:(hf + 1) * 512],
                                lhsT=bd2we2_sb[:], rhs=e1b[:, c0:c0 + 512],
                                start=True, stop=True, tile_position=(0, 64),
                            )
                        dst = e2r[u][:, cp * 1024:(cp + 1) * 1024]
                        if cp in E2RELU_DVE:
                            nc.vector.tensor_scalar(
                                out=dst, in0=ps[:], scalar1=be2_sb[:, 0:1],
                                scalar2=0.0, op0=OP.add, op1=OP.max)
                        else:
                            nc.scalar.activation(
                                out=dst, in_=ps[:], func=AF.Relu,
                                bias=be2_sb[:, 0:1], scale=1.0,
                            )
                    del e1_tiles[2 * u], e1_tiles[2 * u + 1]

            # ================= finals + sigmoid/tanh =================
            with tc.tile_pool(name="ps_fin", bufs=2, space="PSUM") as ps_fin, \
                 tc.tile_pool(name="outstage", bufs=2) as outstage:
                for half in range(2):
                    ps = ps_fin.tile([128, 2048], f32, tag="fin", name="ps_fin_t")
                    for u in range(4):
                        for ch in range(4):
                            c0 = half * 2048 + ch * 512
                            nc.tensor.matmul(
                                ps[32 * u:32 * u + 4, ch * 512:(ch + 1) * 512],
                                lhsT=bd4we3_sb[:], rhs=e2r[u][:, c0:c0 + 512],
                                start=True, stop=True, tile_position=(0, 32 * u),
                            )
                    adj_t = outstage.tile([128, 2048], f32, tag="adj", name="adj_t")
                    nc.scalar.activation(out=adj_t[:], in_=ps[:], func=AF.Sigmoid,
                                         bias=be3_sb[:, 0:1], scale=1.0)
                    for u in range(4):
                        nc.sync.dma_start(
                            out=adj_d[4 * u:4 * u + 4,
                                      half * 2048:(half + 1) * 2048],
                            in_=adj_t[32 * u:32 * u + 4, :])
                for half in range(2):
                    ps = ps_fin.tile([128, 2048], f32, tag="fin", name="ps_fin_t")
                    for gr in range(4):
                        for ch in range(4):
                            c0 = half * 2048 + ch * 512
                            nc.tensor.matmul(
                                ps[32 * gr:32 * gr + 4, ch * 512:(ch + 1) * 512],
                                lhsT=bd4ws2_sb[:], rhs=s1r[gr][:, c0:c0 + 512],
                                start=True, stop=True, tile_position=(0, 32 * gr),
                            )
                    str_t = outstage.tile([128, 2048], f32, tag="strt", name="str_t")
                    nc.scalar.activation(out=str_t[:], in_=ps[:], func=AF.Tanh,
                                         bias=bs2_sb[:, 0:1], scale=1.0)
                    for gr in range(4):
                        nc.sync.dma_start(
                            out=str_d[4 * gr:4 * gr + 4,
                                      half * 2048:(half + 1) * 2048],
                            in_=str_t[32 * gr:32 * gr + 4, :])

    _split_multiwait(nc)
    return nc


def _get_program():
    if "nc" not in _PROGRAM_CACHE:
        _PROGRAM_CACHE["nc"] = _build_program()
    return _PROGRAM_CACHE["nc"]


# ---------------------------------------------------------------------------
# host wrapper
# ---------------------------------------------------------------------------

def kernel(state_sequence, W1, b1, W2, b2, gamma, beta,
           We1, be1, We2, be2, We3, be3, Ws1, bs1, Ws2, bs2):
    global LAST_RESULTS
    from concourse.bass_utils import run_bass_kernel_spmd

    state_sequence = np.asarray(state_sequence, dtype=np.float32)
    W1 = np.asarray(W1, np.float32); b1 = np.asarray(b1, np.float32)
    W2 = np.asarray(W2, np.float32); b2 = np.asarray(b2, np.float32)
    gamma = np.asarray(gamma, np.float32); beta = np.asarray(beta, np.float32)
    We1 = np.asarray(We1, np.float32); be1 = np.asarray(be1, np.float32)
    We2 = np.asarray(We2, np.float32); be2 = np.asarray(be2, np.float32)
    We3 = np.asarray(We3, np.float32); be3 = np.asarray(be3, np.float32)
    Ws1 = np.asarray(Ws1, np.float32); bs1 = np.asarray(bs1, np.float32)
    Ws2 = np.asarray(Ws2, np.float32); bs2 = np.asarray(bs2, np.float32)

    idx = _node_indices()
    nodes = state_sequence[:, idx]                      # [B, N, D]

    def f16(a):
        return np.ascontiguousarray(a.astype(np.float16))

    def f32c(a):
        return np.ascontiguousarray(a.astype(np.float32))

    def blockdiag(w, k):
        m, n = w.shape
        out = np.zeros((m * k, n * k), np.float32)
        for i in range(k):
            out[i * m:(i + 1) * m, i * n:(i + 1) * n] = w
        return out

    shared = {
        "w1": f16(W1), "w2": f16(W2),
        "b1_2": f32c(b1.reshape(2, 128).T),
        "b2c": f32c(b2.reshape(128, 1)),
        "gammap": f32c(gamma.reshape(128, 1)),
        "betap": f32c(beta.reshape(128, 1)),
        "we1a": f16(We1[:128, :]), "we1b": f16(We1[128:, :]),
        "ws1a": f16(Ws1[:128, :]), "ws1b": f16(Ws1[128:, :]),
        "be1_2": f32c(np.tile(be1, 2).reshape(128, 1)),
        "bs1_4": f32c(np.tile(bs1, 4).reshape(128, 1)),
        "be2_4": f32c(np.tile(be2, 4).reshape(128, 1)),
        "bd2we2": f16(blockdiag(We2, 2)),
        "bd4we3": f16(blockdiag(We3, 4)),
        "bd4ws2": f16(blockdiag(Ws2, 4)),
        "be3b": f32c(np.full((128, 1), float(be3[0]))),
        "bs2b": f32c(np.full((128, 1), float(bs2[0]))),
    }

    in_maps = []
    for c in range(NCORES):
        xt = f16(nodes[c * BSH:(c + 1) * BSH].reshape(TOK, D).T)
        in_maps.append({"xt": xt, **shared})

    nc = _get_program()
    res = run_bass_kernel_spmd(nc, in_maps, core_ids=list(range(NCORES)))
    LAST_RESULTS = res

    nf = np.empty((B, N, DM), np.float32)
    adj = np.empty((B, N, N), np.float32)
    stg = np.empty((B, N, N), np.float32)
    mask = (1.0 - np.eye(N, dtype=np.float32))
    for c in range(NCORES):
        r = res.results[c]
        nf_c = r["nf_raw"].reshape(BSH, N, DM) * gamma + beta
        nf[c * BSH:(c + 1) * BSH] = nf_c
        adj[c * BSH:(c + 1) * BSH] = r["adj"].reshape(BSH, N, N) * mask
        stg[c * BSH:(c + 1) * BSH] = r["strg"].reshape(BSH, N, N) * mask
    return nf, adj, stg


# revision 10
# speedup vs baseline: 3.0427x; 3.0427x over previous
"""CausalGraphBuilder Trainium2 kernel.

Full inputs -> shard batch (128) over 8 NeuronCores (16 each) -> Bass kernel
(encoder MLP + LayerNorm + N^2 pair-MLP edge/strength heads) -> gather.

Self-contained: hardcodes B,S,D,N = 128,1024,512,64 and the sharding.
"""

import numpy as np

B, S, D, N = 128, 1024, 512, 64
DM = D // 4          # 128 node-feature dim after encoder
NCORES = 8
BSH = B // NCORES    # 16 batch per core
TOK = BSH * N        # 1024 tokens per core

# jnp.linspace(0, S-1, N).astype(int32) as evaluated by the reference in this
# environment (device cast rounds); recomputed at runtime when jax is available.
_FALLBACK_IDX = [0, 16, 32, 49, 65, 81, 97, 114, 130, 146, 162, 179, 195, 211,
                 227, 244, 260, 276, 292, 309, 325, 341, 357, 373, 390, 406,
                 422, 438, 455, 471, 487, 503, 520, 536, 552, 568, 585, 601,
                 617, 633, 650, 666, 682, 698, 714, 731, 747, 763, 779, 796,
                 812, 828, 844, 861, 877, 893, 909, 926, 942, 958, 974, 991,
                 1007, 1023]


def _node_indices():
    try:
        import jax.numpy as jnp
        idx = np.asarray(jnp.linspace(0.0, float(S - 1), N).astype(jnp.int32))
        if idx.shape == (N,):
            return idx.astype(np.int64)
    except Exception:
        pass
    return np.array(_FALLBACK_IDX, dtype=np.int64)


# ---------------------------------------------------------------------------
# device program
# ---------------------------------------------------------------------------

_PROGRAM_CACHE = {}
LAST_RESULTS = None  # BassKernelResults of the most recent run (for test.py)

# engine assignment knobs (tuned against the profile)
PE_E_TILES = (6,)             # E-groups built on PE (identity-matmul 2-pass)
PE_S_TILES = (0, 1, 2, 3)     # S-groups built on PE
EVAC_DVE_E = ()               # PE-built E tiles evacuated by DVE instead of ACT
EVAC_DVE_S = ()
GP_RELU = False               # DVE-built tiles: relu on GPSIMD (slow ucode!)
E2RELU_DVE = ()               # e2 cp-chunks (0..3) whose relu-evac runs on DVE


def _patched_tile_context(nc):
    """TileContext whose tail drain never carries >1 sync wait (this walrus
    build rejects multi-wait CTRL instructions)."""
    import concourse.mybir as mybir
    import concourse.tile as tile
    from concourse.vector_clock import ScopedClock

    class TileContextP(tile.TileContext):
        def _drain_and_barrier(self, tick_clock, wait_clock):
            drain_inst = self.nc.sync.drain()
            wait_clock.add_sem_waits(
                drain_inst.ins, ScopedClock({None: tick_clock.global_clock})
            )
            si = drain_inst.ins.sync_info
            if si is not None and si.on_wait is not None and len(si.on_wait) > 1:
                waits = list(si.on_wait)
                si.on_wait = waits[:1]
                for w in waits[1:]:
                    extra = self.nc.sync.drain()
                    extra.ins.sync_info = mybir.SyncInfo(on_wait=[w], on_update=[])
            self.nc.all_engine_barrier()
            popped = self.nc._tile_sem_poison_stack.pop()
            assert popped is self._sem_poison
            self.nc.clear_and_free_semaphores(list(self.sems.allocated().values()))
            self.nc.all_engine_barrier()

    return TileContextP(nc)


def _split_multiwait(nc):
    """This walrus build accepts at most ONE sync wait per instruction; hoist
    extra waits into single-wait NoOps on the same engine just before."""
    import concourse.mybir as mybir

    n_split = 0
    for f in nc.m.functions:
        for bb in f.blocks:
            insts = list(bb.instructions)
            out = []
            for ins in insts:
                si = ins.sync_info
                if si is not None and si.on_wait is not None and len(si.on_wait) > 1:
                    waits = list(si.on_wait)
                    for w in waits[:-1]:
                        nop = mybir.InstNoOp(
                            name=f"{ins.name}-w{n_split}",
                            engine=ins.engine,
                            bass_nofuse=True,
                            sync_info=mybir.SyncInfo(on_wait=[w], on_update=[]),
                        )
                        out.append(nop)
                        n_split += 1
                    si.on_wait = waits[-1:]
                out.append(ins)
            if n_split:
                bb.instructions = out
    return n_split


def _build_program():
    import concourse.bass as bass
    import concourse.mybir as mybir
    from concourse.masks import make_identity

    f16 = mybir.dt.float16
    f32 = mybir.dt.float32
    AF = mybir.ActivationFunctionType
    OP = mybir.AluOpType

    nc = bass.Bass()

    # ---- DRAM I/O ----
    xt_d = nc.dram_tensor("xt", [D, TOK], f16, kind="ExternalInput")
    w1_d = nc.dram_tensor("w1", [D, 2 * DM], f16, kind="ExternalInput")
    w2_d = nc.dram_tensor("w2", [2 * DM, DM], f16, kind="ExternalInput")
    b1_d = nc.dram_tensor("b1_2", [128, 2], f32, kind="ExternalInput")
    b2_d = nc.dram_tensor("b2c", [128, 1], f32, kind="ExternalInput")
    gp_d = nc.dram_tensor("gammap", [128, 1], f32, kind="ExternalInput")
    bp_d = nc.dram_tensor("betap", [128, 1], f32, kind="ExternalInput")
    we1a_d = nc.dram_tensor("we1a", [128, 64], f16, kind="ExternalInput")
    we1b_d = nc.dram_tensor("we1b", [128, 64], f16, kind="ExternalInput")
    ws1a_d = nc.dram_tensor("ws1a", [128, 32], f16, kind="ExternalInput")
    ws1b_d = nc.dram_tensor("ws1b", [128, 32], f16, kind="ExternalInput")
    be1_d = nc.dram_tensor("be1_2", [128, 1], f32, kind="ExternalInput")
    bs1_d = nc.dram_tensor("bs1_4", [128, 1], f32, kind="ExternalInput")
    be2_d = nc.dram_tensor("be2_4", [128, 1], f32, kind="ExternalInput")
    bd2we2_d = nc.dram_tensor("bd2we2", [128, 64], f16, kind="ExternalInput")
    bd4we3_d = nc.dram_tensor("bd4we3", [128, 4], f16, kind="ExternalInput")
    bd4ws2_d = nc.dram_tensor("bd4ws2", [128, 4], f16, kind="ExternalInput")
    be3_d = nc.dram_tensor("be3b", [128, 1], f32, kind="ExternalInput")
    bs2_d = nc.dram_tensor("bs2b", [128, 1], f32, kind="ExternalInput")

    nf_d = nc.dram_tensor("nf_raw", [TOK, DM], f32, kind="ExternalOutput")
    adj_d = nc.dram_tensor("adj", [BSH, N * N], f32, kind="ExternalOutput")
    str_d = nc.dram_tensor("strg", [BSH, N * N], f32, kind="ExternalOutput")

    with _patched_tile_context(nc) as tc:
        from contextlib import ExitStack

        with ExitStack() as ctx:
            consts = ctx.enter_context(tc.tile_pool(name="consts", bufs=1))
            persist = ctx.enter_context(tc.tile_pool(name="persist", bufs=1))

            # ---- load constants ----
            def load(pool, name, dram, shape, dtype):
                t = pool.tile(shape, dtype, tag=name, name=name)
                nc.sync.dma_start(out=t[:], in_=dram[:])
                return t

            xt_sb = [load(consts, f"xt{c}", xt_d[c * 128:(c + 1) * 128, :],
                          [128, TOK], f16) for c in range(4)]
            w1_sb = [load(consts, f"w1_{c}", w1_d[c * 128:(c + 1) * 128, :],
                          [128, 2 * DM], f16) for c in range(4)]
            w2_sb = [load(consts, f"w2_{c}", w2_d[c * 128:(c + 1) * 128, :],
                          [128, DM], f16) for c in range(2)]
            b1_sb = load(consts, "b1", b1_d, [128, 2], f32)
            b2_sb = load(consts, "b2", b2_d, [128, 1], f32)
            gp_sb = load(consts, "gp", gp_d, [128, 1], f32)
            bp_sb = load(consts, "bp", bp_d, [128, 1], f32)
            we1a_sb = load(consts, "we1a", we1a_d, [128, 64], f16)
            we1b_sb = load(consts, "we1b", we1b_d, [128, 64], f16)
            ws1a_sb = load(consts, "ws1a", ws1a_d, [128, 32], f16)
            ws1b_sb = load(consts, "ws1b", ws1b_d, [128, 32], f16)
            be1_sb = load(consts, "be1", be1_d, [128, 1], f32)
            bs1_sb = load(consts, "bs1", bs1_d, [128, 1], f32)
            be2_sb = load(consts, "be2", be2_d, [128, 1], f32)
            bd2we2_sb = load(consts, "bd2we2", bd2we2_d, [128, 64], f16)
            bd4we3_sb = load(consts, "bd4we3", bd4we3_d, [128, 4], f16)
            bd4ws2_sb = load(consts, "bd4ws2", bd4ws2_d, [128, 4], f16)
            be3_sb = load(consts, "be3", be3_d, [128, 1], f32)
            bs2_sb = load(consts, "bs2", bs2_d, [128, 1], f32)

            eps_sb = consts.tile([128, 1], f32, tag="eps")
            nc.vector.memset(eps_sb[:], 1e-5)

            idf32 = consts.tile([128, 128], f32, tag="idf32")
            make_identity(nc, idf32[:])
            idf16 = consts.tile([128, 128], f16, tag="idf16")
            make_identity(nc, idf16[:])

            h1t = [persist.tile([128, TOK], f16, tag=f"h1t{m}", name=f"h1t{m}") for m in range(2)]
            ht = persist.tile([128, TOK], f32, tag="ht")
            nfraw = persist.tile([128, 8, 128], f32, tag="nfraw")
            nft = persist.tile([128, TOK], f16, tag="nft")
            ab = [persist.tile([128, 128], f16, tag=f"ab{g}", name=f"ab{g}") for g in range(8)]
            apbp = [persist.tile([128, 128], f16, tag=f"apbp{gr}", name=f"apbp{gr}") for gr in range(4)]
            e2r = [persist.tile([128, N * N], f16, tag=f"e2r{u}", name=f"e2r{u}") for u in range(4)]
            s1r = [persist.tile([128, N * N], f16, tag=f"s1r{gr}", name=f"s1r{gr}") for gr in range(4)]

            # ================= encoder =================
            with tc.tile_pool(name="ps_enc", bufs=2, space="PSUM") as ps_enc:
                for m in range(2):
                    for n2 in range(2):
                        ps = ps_enc.tile([128, 512], f32, tag="enc", name="ps_enc_t")
                        for c in range(4):
                            nc.tensor.matmul(
                                ps[:],
                                lhsT=w1_sb[c][:, m * 128:(m + 1) * 128],
                                rhs=xt_sb[c][:, n2 * 512:(n2 + 1) * 512],
                                start=(c == 0), stop=(c == 3),
                            )
                        nc.scalar.activation(
                            out=h1t[m][:, n2 * 512:(n2 + 1) * 512], in_=ps[:],
                            func=AF.Relu, bias=b1_sb[:, m:m + 1], scale=1.0,
                        )
                for n2 in range(2):
                    ps = ps_enc.tile([128, 512], f32, tag="enc", name="ps_enc_t")
                    for c2 in range(2):
                        nc.tensor.matmul(
                            ps[:], lhsT=w2_sb[c2][:],
                            rhs=h1t[c2][:, n2 * 512:(n2 + 1) * 512],
                            start=(c2 == 0), stop=(c2 == 1),
                        )
                    nc.scalar.activation(
                        out=ht[:, n2 * 512:(n2 + 1) * 512], in_=ps[:],
                        func=AF.Identity, bias=b2_sb[:, 0:1], scale=1.0,
                    )

            # ================= layernorm (token-major) =================
            with tc.tile_pool(name="ps_tr", bufs=3, space="PSUM") as ps_tr, \
                 tc.tile_pool(name="ln_tmp", bufs=4) as ln_tmp:
                for t in range(8):
                    pst = ps_tr.tile([128, 128], f32, tag="htok", name="pst")
                    nc.tensor.transpose(pst[:], ht[:, t * 128:(t + 1) * 128],
                                        idf32[:])
                    st6 = ln_tmp.tile([128, 6], f32, tag="st6", name="st6")
                    nc.vector.bn_stats(out=st6[:], in_=pst[:])
                    mv = ln_tmp.tile([128, 2], f32, tag="mv", name="mv")
                    nc.vector.bn_aggr(out=mv[:], in_=st6[:])
                    rstd = ln_tmp.tile([128, 1], f32, tag="rstd", name="rstd")
                    nc.scalar.activation(out=rstd[:], in_=mv[:, 1:2],
                                         func=AF.Sqrt, bias=eps_sb[:, 0:1],
                                         scale=1.0)
                    nc.vector.reciprocal(out=rstd[:], in_=rstd[:])
                    nc.vector.tensor_scalar(
                        out=nfraw[:, t, :], in0=pst[:],
                        scalar1=mv[:, 0:1], scalar2=rstd[:, 0:1],
                        op0=OP.subtract, op1=OP.mult,
                    )
                    nc.sync.dma_start(out=nf_d[t * 128:(t + 1) * 128, :],
                                      in_=nfraw[:, t, :])
                # nfT (feature-major) with gamma/beta applied per-partition
                for t in range(8):
                    psn = ps_tr.tile([128, 128], f32, tag="nft_ps", name="psn")
                    nc.tensor.transpose(psn[:], nfraw[:, t, :], idf32[:])
                    nc.vector.tensor_scalar(
                        out=nft[:, t * 128:(t + 1) * 128], in0=psn[:],
                        scalar1=gp_sb[:, 0:1], scalar2=bp_sb[:, 0:1],
                        op0=OP.mult, op1=OP.add,
                    )

            # ================= pair projections =================
            with tc.tile_pool(name="ps_proj", bufs=3, space="PSUM") as ps_proj:
                for g in range(8):           # edge groups: b = 2g, 2g+1
                    ps = ps_proj.tile([128, 128], f32, tag="proj_e", name="ps_proj_e")
                    for bb in range(2):
                        b = 2 * g + bb
                        cols = nft[:, b * N:(b + 1) * N]
                        nc.tensor.matmul(
                            ps[bb * 64:(bb + 1) * 64, 0:64], lhsT=we1a_sb[:],
                            rhs=cols, start=True, stop=True,
                            tile_position=(0, 64 * bb),
                        )
                        nc.tensor.matmul(
                            ps[bb * 64:(bb + 1) * 64, 64:128], lhsT=we1b_sb[:],
                            rhs=cols, start=True, stop=True,
                            tile_position=(0, 64 * bb),
                        )
                    nc.scalar.activation(out=ab[g][:], in_=ps[:], func=AF.Copy)
                for gr in range(4):          # strength groups: b = 4gr..4gr+3
                    ps = ps_proj.tile([128, 128], f32, tag="proj_s", name="ps_proj_s")
                    for bb in range(4):
                        b = 4 * gr + bb
                        cols = nft[:, b * N:(b + 1) * N]
                        nc.tensor.matmul(
                            ps[bb * 32:(bb + 1) * 32, 0:64], lhsT=ws1a_sb[:],
                            rhs=cols, start=True, stop=True,
                            tile_position=(0, 32 * bb),
                        )
                        nc.tensor.matmul(
                            ps[bb * 32:(bb + 1) * 32, 64:128], lhsT=ws1b_sb[:],
                            rhs=cols, start=True, stop=True,
                            tile_position=(0, 32 * bb),
                        )
                    nc.scalar.activation(out=apbp[gr][:], in_=ps[:], func=AF.Copy)

            # ================= N^2 builds + edge layer2 =================
            with tc.tile_pool(name="e1pool", bufs=3) as e1pool, \
                 tc.tile_pool(name="ps_bld", bufs=2, space="PSUM") as ps_bld, \
                 tc.tile_pool(name="ps_e2", bufs=2, space="PSUM") as ps_e2:
                e1_tiles = {}

                def build_pe(out_flat, src, bias, evac_dve):
                    """2-pass identity-matmul broadcast build + relu evac.

                    src: [128, 128] sbuf tile, A in cols 0:64, B in cols 64:128.
                    out_flat: [128, 4096] fp16 destination (relu'd)."""
                    for q in range(4):           # 1024-pair chunks (16 i vals)
                        ps = ps_bld.tile([128, 1024], f32, tag="bld",
                                         name="ps_bld_t")
                        for hf in range(2):      # 512-col matmuls (8 i vals)
                            i0 = q * 16 + hf * 8
                            a_chunk = src[:, i0:i0 + 8][:, :, None] \
                                .broadcast_to((128, 8, 64))
                            b_chunk = src[:, None, 64:128] \
                                .broadcast_to((128, 8, 64))
                            o = ps[:, hf * 512:(hf + 1) * 512]
                            nc.tensor.matmul(o, lhsT=idf16[:], rhs=a_chunk,
                                             start=True, stop=False)
                            nc.tensor.matmul(o, lhsT=idf16[:], rhs=b_chunk,
                                             start=False, stop=True)
                        dst = out_flat[:, q * 1024:(q + 1) * 1024]
                        if evac_dve:
                            nc.vector.tensor_scalar(
                                out=dst, in0=ps[:], scalar1=bias,
                                scalar2=0.0, op0=OP.add, op1=OP.max)
                        else:
                            nc.scalar.activation(out=dst, in_=ps[:],
                                                 func=AF.Relu, bias=bias,
                                                 scale=1.0)

                def build_dve(out3, src, bias):
                    a_view = src[:, 0:64][:, :, None].broadcast_to((128, N, N))
                    b_view = src[:, None, 64:128].broadcast_to((128, N, N))
                    nc.vector.scalar_tensor_tensor(
                        out=out3, in0=b_view, scalar=bias,
                        in1=a_view, op0=OP.add, op1=OP.add,
                    )
                    relu_eng = nc.gpsimd if GP_RELU else nc.vector
                    relu_eng.tensor_scalar_max(out=out3, in0=out3, scalar1=0.0)

                def build_edge(g):
                    e1 = e1pool.tile([128, N, N], f16, tag="e1", name=f"e1_{g}")
                    if g in PE_E_TILES:
                        build_pe(e1[:].rearrange("p i j -> p (i j)"), ab[g][:],
                                 be1_sb[:, 0:1], g in EVAC_DVE_E)
                    else:
                        build_dve(e1[:], ab[g][:], be1_sb[:, 0:1])
                    e1_tiles[g] = e1

                def build_strength(gr):
                    s1 = s1r[gr]
                    s3 = s1[:].rearrange("p (i j) -> p i j", i=N)
                    if gr in PE_S_TILES:
                        build_pe(s1[:], apbp[gr][:], bs1_sb[:, 0:1],
                                 gr in EVAC_DVE_S)
                    else:
                        build_dve(s3, apbp[gr][:], bs1_sb[:, 0:1])

                for u in range(4):
                    build_edge(2 * u)
                    build_edge(2 * u + 1)
                    build_strength(u)
                    e1a = e1_tiles[2 * u][:].rearrange("p i j -> p (i j)")
                    e1b = e1_tiles[2 * u + 1][:].rearrange("p i j -> p (i j)")
                    for cp in range(4):
                        ps = ps_e2.tile([128, 1024], f32, tag="e2", name="ps_e2_t")
                        for hf in range(2):
                            c0 = cp * 1024 + hf * 512
                            nc.tensor.matmul(
                                ps[0:64, hf * 512:(hf + 1) * 512],
                                lhsT=bd2we2_sb[:], rhs=e1a[:, c0:c0 + 512],
                                start=True, stop=True, tile_position=(0, 0),
                            )
                            nc.tensor.matmul(
                                ps[64:128, hf * 512:(hf + 1) * 512],
                                lhsT=bd2we2_sb[:], rhs=e1b[:, c0:c0 + 512],
                                start=True, stop=True, tile_position=(0, 64),
                            )
                        dst = e2r[u][:, cp * 1024:(cp + 1) * 1024]
                        if cp in E2RELU_DVE:
                            nc.vector.tensor_scalar(
                                out=dst, in0=ps[:], scalar1=be2_sb[:, 0:1],
                                scalar2=0.0, op0=OP.add, op1=OP.max)
                        else:
                            nc.scalar.activation(
                                out=dst, in_=ps[:], func=AF.Relu,
                                bias=be2_sb[:, 0:1], scale=1.0,
                            )
                    del e1_tiles[2 * u], e1_tiles[2 * u + 1]

            # ================= finals + sigmoid/tanh =================
            with tc.tile_pool(name="ps_fin", bufs=2, space="PSUM") as ps_fin, \
                 tc.tile_pool(name="outstage", bufs=2) as outstage:
                for half in range(2):
                    ps = ps_fin.tile([128, 2048], f32, tag="fin", name="ps_fin_t")
                    for u in range(4):
                        for ch in range(4):
                            c0 = half * 2048 + ch * 512
                            nc.tensor.matmul(
                                ps[32 * u:32 * u + 4, ch * 512:(ch + 1) * 512],
                                lhsT=bd4we3_sb[:], rhs=e2r[u][:, c0:c0 + 512],
                                start=True, stop=True, tile_position=(0, 32 * u),
                            )
                    adj_t = outstage.tile([128, 2048], f32, tag="adj", name="adj_t")
                    nc.scalar.activation(out=adj_t[:], in_=ps[:], func=AF.Sigmoid,
                                         bias=be3_sb[:, 0:1], scale=1.0)
                    for u in range(4):
                        nc.sync.dma_start(
                            out=adj_d[4 * u:4 * u + 4,
                                      half * 2048:(half + 1) * 2048],
                            in_=adj_t[32 * u:32 * u + 4, :])
                for half in range(2):
                    ps = ps_fin.tile([128, 2048], f32, tag="fin", name="ps_fin_t")
                    for gr in range(4):
                        for ch in range(4):
                            c0 = half * 2048 + ch * 512
                            nc.tensor.matmul(
                                ps[32 * gr:32 * gr + 4, ch * 512:(ch + 1) * 512],
                                lhsT=bd4ws2_sb[:], rhs=s1r[gr][:, c0:c0 + 512],
                                start=True, stop=True, tile_position=(0, 32 * gr),
                            )
                    str_t = outstage.tile([128, 2048], f32, tag="strt", name="str_t")
                    nc.scalar.activation(out=str_t[:], in_=ps[:], func=AF.Tanh,
                                         bias=bs2_sb[:, 0:1], scale=1.0)
                    for gr in range(4):
                        nc.sync.dma_start(
                            out=str_d[4 * gr:4 * gr + 4,
                                      half * 2048:(half + 1) * 2048],
                            in_=str_t[32 * gr:32 * gr + 4, :])

    _split_multiwait(nc)
    return nc


def _get_program():
    if "nc" not in _PROGRAM_CACHE:
        _PROGRAM_CACHE["nc"] = _build_program()
    return _PROGRAM_CACHE["nc"]


# ---------------------------------------------------------------------------
# host wrapper
# ---------------------------------------------------------------------------

def kernel(state_sequence, W1, b1, W2, b2, gamma, beta,
           We1, be1, We2, be2, We3, be3, Ws1, bs1, Ws2, bs2):
    global LAST_RESULTS
    from concourse.bass_utils import run_bass_kernel_spmd

    state_sequence = np.asarray(state_sequence, dtype=np.float32)
    W1 = np.asarray(W1, np.float32); b1 = np.asarray(b1, np.float32)
    W2 = np.asarray(W2, np.float32); b2 = np.asarray(b2, np.float32)
    gamma = np.asarray(gamma, np.float32); beta = np.asarray(beta, np.float32)
    We1 = np.asarray(We1, np.float32); be1 = np.asarray(be1, np.float32)
    We2 = np.asarray(We2, np.float32); be2 = np.asarray(be2, np.float32)
    We3 = np.asarray(We3, np.float32); be3 = np.asarray(be3, np.float32)
    Ws1 = np.asarray(Ws1, np.float32); bs1 = np.asarray(bs1, np.float32)
    Ws2 = np.asarray(Ws2, np.float32); bs2 = np.asarray(bs2, np.float32)

    idx = _node_indices()
    nodes = state_sequence[:, idx]                      # [B, N, D]

    def f16(a):
        return np.ascontiguousarray(a.astype(np.float16))

    def f32c(a):
        return np.ascontiguousarray(a.astype(np.float32))

    def blockdiag(w, k):
        m, n = w.shape
        out = np.zeros((m * k, n * k), np.float32)
        for i in range(k):
            out[i * m:(i + 1) * m, i * n:(i + 1) * n] = w
        return out

    shared = {
        "w1": f16(W1), "w2": f16(W2),
        "b1_2": f32c(b1.reshape(2, 128).T),
        "b2c": f32c(b2.reshape(128, 1)),
        "gammap": f32c(gamma.reshape(128, 1)),
        "betap": f32c(beta.reshape(128, 1)),
        "we1a": f16(We1[:128, :]), "we1b": f16(We1[128:, :]),
        "ws1a": f16(Ws1[:128, :]), "ws1b": f16(Ws1[128:, :]),
        "be1_2": f32c(np.tile(be1, 2).reshape(128, 1)),
        "bs1_4": f32c(np.tile(bs1, 4).reshape(128, 1)),
        "be2_4": f32c(np.tile(be2, 4).reshape(128, 1)),
        "bd2we2": f16(blockdiag(We2, 2)),
        "bd4we3": f16(blockdiag(We3, 4)),
        "bd4ws2": f16(blockdiag(Ws2, 4)),
        "be3b": f32c(np.full((128, 1), float(be3[0]))),
        "bs2b": f32c(np.full((128, 1), float(bs2[0]))),
    }

    in_maps = []
    for c in range(NCORES):
        xt = f16(nodes[c * BSH:(c + 1) * BSH].reshape(TOK, D).T)
        in_maps.append({"xt": xt, **shared})

    nc = _get_program()
    res = run_bass_kernel_spmd(nc, in_maps, core_ids=list(range(NCORES)))
    LAST_RESULTS = res

    nf = np.empty((B, N, DM), np.float32)
    adj = np.empty((B, N, N), np.float32)
    stg = np.empty((B, N, N), np.float32)
    mask = (1.0 - np.eye(N, dtype=np.float32))
    for c in range(NCORES):
        r = res.results[c]
        nf_c = r["nf_raw"].reshape(BSH, N, DM) * gamma + beta
        nf[c * BSH:(c + 1) * BSH] = nf_c
        adj[c * BSH:(c + 1) * BSH] = r["adj"].reshape(BSH, N, N) * mask
        stg[c * BSH:(c + 1) * BSH] = r["strg"].reshape(BSH, N, N) * mask
    return nf, adj, stg


# revision 11
# speedup vs baseline: 3.0766x; 1.0112x over previous
"""CausalGraphBuilder Trainium2 kernel.

Full inputs -> shard batch (128) over 8 NeuronCores (16 each) -> Bass kernel
(encoder MLP + LayerNorm + N^2 pair-MLP edge/strength heads) -> gather.

Self-contained: hardcodes B,S,D,N = 128,1024,512,64 and the sharding.
"""

import numpy as np

B, S, D, N = 128, 1024, 512, 64
DM = D // 4          # 128 node-feature dim after encoder
NCORES = 8
BSH = B // NCORES    # 16 batch per core
TOK = BSH * N        # 1024 tokens per core

# jnp.linspace(0, S-1, N).astype(int32) as evaluated by the reference in this
# environment (device cast rounds); recomputed at runtime when jax is available.
_FALLBACK_IDX = [0, 16, 32, 49, 65, 81, 97, 114, 130, 146, 162, 179, 195, 211,
                 227, 244, 260, 276, 292, 309, 325, 341, 357, 373, 390, 406,
                 422, 438, 455, 471, 487, 503, 520, 536, 552, 568, 585, 601,
                 617, 633, 650, 666, 682, 698, 714, 731, 747, 763, 779, 796,
                 812, 828, 844, 861, 877, 893, 909, 926, 942, 958, 974, 991,
                 1007, 1023]


def _node_indices():
    try:
        import jax.numpy as jnp
        idx = np.asarray(jnp.linspace(0.0, float(S - 1), N).astype(jnp.int32))
        if idx.shape == (N,):
            return idx.astype(np.int64)
    except Exception:
        pass
    return np.array(_FALLBACK_IDX, dtype=np.int64)


# ---------------------------------------------------------------------------
# device program
# ---------------------------------------------------------------------------

_PROGRAM_CACHE = {}
LAST_RESULTS = None  # BassKernelResults of the most recent run (for test.py)

# engine assignment knobs (tuned against the profile)
PE_E_TILES = (6,)             # E-groups built on PE (identity-matmul 2-pass)
PE_S_TILES = (0, 1, 2, 3)     # S-groups built on PE
EVAC_DVE_E = ()               # PE-built E tiles evacuated by DVE instead of ACT
EVAC_DVE_S = ()
GP_RELU = False               # DVE-built tiles: relu on GPSIMD (slow ucode!)
E2RELU_DVE = ()               # e2 cp-chunks (0..3) whose relu-evac runs on DVE



# packed-constant blob layouts (columns)
_W16_SECTIONS = [
    ("w1_0", 256), ("w1_1", 256), ("w1_2", 256), ("w1_3", 256),
    ("w2_0", 128), ("w2_1", 128),
    ("we1a", 64), ("we1b", 64), ("ws1a", 32), ("ws1b", 32),
    ("bd2we2", 64), ("bd4we3", 4), ("bd4ws2", 4),
]
W16_OFF = {}
_o = 0
for _k, _w in _W16_SECTIONS:
    W16_OFF[_k] = (_o, _w)
    _o += _w
W16_COLS = _o
_W32_SECTIONS = [("b1_2", 2), ("b2c", 1), ("gammap", 1), ("betap", 1),
                 ("be1_2", 1), ("bs1_4", 1), ("be2_4", 1), ("be3b", 1),
                 ("bs2b", 1)]
W32_OFF = {}
_o = 0
for _k, _w in _W32_SECTIONS:
    W32_OFF[_k] = _o
    _o += _w
W32_COLS = _o

def _patched_tile_context(nc):
    """TileContext whose tail drain never carries >1 sync wait (this walrus
    build rejects multi-wait CTRL instructions)."""
    import concourse.mybir as mybir
    import concourse.tile as tile
    from concourse.vector_clock import ScopedClock

    class TileContextP(tile.TileContext):
        def _drain_and_barrier(self, tick_clock, wait_clock):
            drain_inst = self.nc.sync.drain()
            wait_clock.add_sem_waits(
                drain_inst.ins, ScopedClock({None: tick_clock.global_clock})
            )
            si = drain_inst.ins.sync_info
            if si is not None and si.on_wait is not None and len(si.on_wait) > 1:
                waits = list(si.on_wait)
                si.on_wait = waits[:1]
                for w in waits[1:]:
                    extra = self.nc.sync.drain()
                    extra.ins.sync_info = mybir.SyncInfo(on_wait=[w], on_update=[])
            self.nc.all_engine_barrier()
            popped = self.nc._tile_sem_poison_stack.pop()
            assert popped is self._sem_poison
            self.nc.clear_and_free_semaphores(list(self.sems.allocated().values()))
            self.nc.all_engine_barrier()

    return TileContextP(nc)


def _split_multiwait(nc):
    """This walrus build accepts at most ONE sync wait per instruction; hoist
    extra waits into single-wait NoOps on the same engine just before."""
    import concourse.mybir as mybir

    n_split = 0
    for f in nc.m.functions:
        for bb in f.blocks:
            insts = list(bb.instructions)
            out = []
            for ins in insts:
                si = ins.sync_info
                if si is not None and si.on_wait is not None and len(si.on_wait) > 1:
                    waits = list(si.on_wait)
                    for w in waits[:-1]:
                        nop = mybir.InstNoOp(
                            name=f"{ins.name}-w{n_split}",
                            engine=ins.engine,
                            bass_nofuse=True,
                            sync_info=mybir.SyncInfo(on_wait=[w], on_update=[]),
                        )
                        out.append(nop)
                        n_split += 1
                    si.on_wait = waits[-1:]
                out.append(ins)
            if n_split:
                bb.instructions = out
    return n_split


def _build_program():
    import concourse.bass as bass
    import concourse.mybir as mybir
    from concourse.masks import make_identity

    f16 = mybir.dt.float16
    f32 = mybir.dt.float32
    AF = mybir.ActivationFunctionType
    OP = mybir.AluOpType

    nc = bass.Bass()

    # ---- DRAM I/O ----
    xt_d = nc.dram_tensor("xt", [D, TOK], f16, kind="ExternalInput")
    wb16_d = nc.dram_tensor("wb16", [128, W16_COLS], f16, kind="ExternalInput")
    wb32_d = nc.dram_tensor("wb32", [128, W32_COLS], f32, kind="ExternalInput")

    nf_d = nc.dram_tensor("nf_raw", [TOK, DM], f32, kind="ExternalOutput")
    adj_d = nc.dram_tensor("adj", [BSH, N * N], f32, kind="ExternalOutput")
    str_d = nc.dram_tensor("strg", [BSH, N * N], f32, kind="ExternalOutput")

    with _patched_tile_context(nc) as tc:
        from contextlib import ExitStack

        with ExitStack() as ctx:
            consts = ctx.enter_context(tc.tile_pool(name="consts", bufs=1))
            persist = ctx.enter_context(tc.tile_pool(name="persist", bufs=1))

            # ---- load constants ----
            def load(pool, name, dram, shape, dtype):
                t = pool.tile(shape, dtype, tag=name, name=name)
                nc.sync.dma_start(out=t[:], in_=dram[:])
                return t

            xt_sb = [load(consts, f"xt{c}", xt_d[c * 128:(c + 1) * 128, :],
                          [128, TOK], f16) for c in range(4)]
            wb16 = load(consts, "wb16", wb16_d, [128, W16_COLS], f16)
            wb32 = load(consts, "wb32", wb32_d, [128, W32_COLS], f32)

            def c16(key):
                o, w = W16_OFF[key]
                return wb16[:, o:o + w]

            def c32(key):
                o = W32_OFF[key]
                return wb32[:, o:o + 1]

            w1_sb = [c16(f"w1_{c}") for c in range(4)]
            w2_sb = [c16(f"w2_{c}") for c in range(2)]
            we1a_sb = c16("we1a"); we1b_sb = c16("we1b")
            ws1a_sb = c16("ws1a"); ws1b_sb = c16("ws1b")
            bd2we2_sb = c16("bd2we2")
            bd4we3_sb = c16("bd4we3"); bd4ws2_sb = c16("bd4ws2")
            b1_sb = wb32[:, W32_OFF["b1_2"]:W32_OFF["b1_2"] + 2]
            b2_sb = c32("b2c"); gp_sb = c32("gammap"); bp_sb = c32("betap")
            be1_sb = c32("be1_2"); bs1_sb = c32("bs1_4"); be2_sb = c32("be2_4")
            be3_sb = c32("be3b"); bs2_sb = c32("bs2b")

            eps_sb = consts.tile([128, 1], f32, tag="eps")
            nc.vector.memset(eps_sb[:], 1e-5)

            idf32 = consts.tile([128, 128], f32, tag="idf32")
            make_identity(nc, idf32[:])
            idf16 = consts.tile([128, 128], f16, tag="idf16")
            make_identity(nc, idf16[:])

            h1t = [persist.tile([128, TOK], f16, tag=f"h1t{m}", name=f"h1t{m}") for m in range(2)]
            ht = persist.tile([128, TOK], f32, tag="ht")
            nfraw = persist.tile([128, 8, 128], f32, tag="nfraw")
            nft = persist.tile([128, TOK], f16, tag="nft")
            ab = [persist.tile([128, 128], f16, tag=f"ab{g}", name=f"ab{g}") for g in range(8)]
            apbp = [persist.tile([128, 128], f16, tag=f"apbp{gr}", name=f"apbp{gr}") for gr in range(4)]
            e2r = [persist.tile([128, N * N], f16, tag=f"e2r{u}", name=f"e2r{u}") for u in range(4)]
            s1r = [persist.tile([128, N * N], f16, tag=f"s1r{gr}", name=f"s1r{gr}") for gr in range(4)]

            # ================= encoder =================
            with tc.tile_pool(name="ps_enc", bufs=2, space="PSUM") as ps_enc:
                for m in range(2):
                    for n2 in range(2):
                        ps = ps_enc.tile([128, 512], f32, tag="enc", name="ps_enc_t")
                        for c in range(4):
                            nc.tensor.matmul(
                                ps[:],
                                lhsT=w1_sb[c][:, m * 128:(m + 1) * 128],
                                rhs=xt_sb[c][:, n2 * 512:(n2 + 1) * 512],
                                start=(c == 0), stop=(c == 3),
                            )
                        nc.scalar.activation(
                            out=h1t[m][:, n2 * 512:(n2 + 1) * 512], in_=ps[:],
                            func=AF.Relu, bias=b1_sb[:, m:m + 1], scale=1.0,
                        )
                for n2 in range(2):
                    ps = ps_enc.tile([128, 512], f32, tag="enc", name="ps_enc_t")
                    for c2 in range(2):
                        nc.tensor.matmul(
                            ps[:], lhsT=w2_sb[c2][:],
                            rhs=h1t[c2][:, n2 * 512:(n2 + 1) * 512],
                            start=(c2 == 0), stop=(c2 == 1),
                        )
                    nc.scalar.activation(
                        out=ht[:, n2 * 512:(n2 + 1) * 512], in_=ps[:],
                        func=AF.Identity, bias=b2_sb[:, 0:1], scale=1.0,
                    )

            # ================= layernorm (token-major) =================
            with tc.tile_pool(name="ps_tr", bufs=3, space="PSUM") as ps_tr, \
                 tc.tile_pool(name="ln_tmp", bufs=4) as ln_tmp:
                for t in range(8):
                    pst = ps_tr.tile([128, 128], f32, tag="htok", name="pst")
                    nc.tensor.transpose(pst[:], ht[:, t * 128:(t + 1) * 128],
                                        idf32[:])
                    st6 = ln_tmp.tile([128, 6], f32, tag="st6", name="st6")
                    nc.vector.bn_stats(out=st6[:], in_=pst[:])
                    mv = ln_tmp.tile([128, 2], f32, tag="mv", name="mv")
                    nc.vector.bn_aggr(out=mv[:], in_=st6[:])
                    rstd = ln_tmp.tile([128, 1], f32, tag="rstd", name="rstd")
                    nc.scalar.activation(out=rstd[:], in_=mv[:, 1:2],
                                         func=AF.Sqrt, bias=eps_sb[:, 0:1],
                                         scale=1.0)
                    nc.vector.reciprocal(out=rstd[:], in_=rstd[:])
                    nc.vector.tensor_scalar(
                        out=nfraw[:, t, :], in0=pst[:],
                        scalar1=mv[:, 0:1], scalar2=rstd[:, 0:1],
                        op0=OP.subtract, op1=OP.mult,
                    )
                    nc.sync.dma_start(out=nf_d[t * 128:(t + 1) * 128, :],
                                      in_=nfraw[:, t, :])
                # nfT (feature-major) with gamma/beta applied per-partition
                for t in range(8):
                    psn = ps_tr.tile([128, 128], f32, tag="nft_ps", name="psn")
                    nc.tensor.transpose(psn[:], nfraw[:, t, :], idf32[:])
                    nc.vector.tensor_scalar(
                        out=nft[:, t * 128:(t + 1) * 128], in0=psn[:],
                        scalar1=gp_sb[:, 0:1], scalar2=bp_sb[:, 0:1],
                        op0=OP.mult, op1=OP.add,
                    )

            # ================= pair projections =================
            with tc.tile_pool(name="ps_proj", bufs=3, space="PSUM") as ps_proj:
                for g in range(8):           # edge groups: b = 2g, 2g+1
                    ps = ps_proj.tile([128, 128], f32, tag="proj_e", name="ps_proj_e")
                    for bb in range(2):
                        b = 2 * g + bb
                        cols = nft[:, b * N:(b + 1) * N]
                        nc.tensor.matmul(
                            ps[bb * 64:(bb + 1) * 64, 0:64], lhsT=we1a_sb[:],
                            rhs=cols, start=True, stop=True,
                            tile_position=(0, 64 * bb),
                        )
                        nc.tensor.matmul(
                            ps[bb * 64:(bb + 1) * 64, 64:128], lhsT=we1b_sb[:],
                            rhs=cols, start=True, stop=True,
                            tile_position=(0, 64 * bb),
                        )
                    nc.scalar.activation(out=ab[g][:], in_=ps[:], func=AF.Copy)
                for gr in range(4):          # strength groups: b = 4gr..4gr+3
                    ps = ps_proj.tile([128, 128], f32, tag="proj_s", name="ps_proj_s")
                    for bb in range(4):
                        b = 4 * gr + bb
                        cols = nft[:, b * N:(b + 1) * N]
                        nc.tensor.matmul(
                            ps[bb * 32:(bb + 1) * 32, 0:64], lhsT=ws1a_sb[:],
                            rhs=cols, start=True, stop=True,
                            tile_position=(0, 32 * bb),
                        )
                        nc.tensor.matmul(
                            ps[bb * 32:(bb + 1) * 32, 64:128], lhsT=ws1b_sb[:],
                            rhs=cols, start=True, stop=True,
                            tile_position=(0, 32 * bb),
                        )
                    nc.scalar.activation(out=apbp[gr][:], in_=ps[:], func=AF.Copy)

            # ================= N^2 builds + edge layer2 =================
            with tc.tile_pool(name="e1pool", bufs=3) as e1pool, \
                 tc.tile_pool(name="ps_bld", bufs=2, space="PSUM") as ps_bld, \
                 tc.tile_pool(name="ps_e2", bufs=2, space="PSUM") as ps_e2:
                e1_tiles = {}

                def build_pe(out_flat, src, bias, evac_dve):
                    """2-pass identity-matmul broadcast build + relu evac.

                    src: [128, 128] sbuf tile, A in cols 0:64, B in cols 64:128.
                    out_flat: [128, 4096] fp16 destination (relu'd)."""
                    for q in range(4):           # 1024-pair chunks (16 i vals)
                        ps = ps_bld.tile([128, 1024], f32, tag="bld",
                                         name="ps_bld_t")
                        for hf in range(2):      # 512-col matmuls (8 i vals)
                            i0 = q * 16 + hf * 8
                            a_chunk = src[:, i0:i0 + 8][:, :, None] \
                                .broadcast_to((128, 8, 64))
                            b_chunk = src[:, None, 64:128] \
                                .broadcast_to((128, 8, 64))
                            o = ps[:, hf * 512:(hf + 1) * 512]
                            nc.tensor.matmul(o, lhsT=idf16[:], rhs=a_chunk,
                                             start=True, stop=False)
                            nc.tensor.matmul(o, lhsT=idf16[:], rhs=b_chunk,
                                             start=False, stop=True)
                        dst = out_flat[:, q * 1024:(q + 1) * 1024]
                        if evac_dve:
                            nc.vector.tensor_scalar(
                                out=dst, in0=ps[:], scalar1=bias,
                                scalar2=0.0, op0=OP.add, op1=OP.max)
                        else:
                            nc.scalar.activation(out=dst, in_=ps[:],
                                                 func=AF.Relu, bias=bias,
                                                 scale=1.0)

                def build_dve(out3, src, bias):
                    a_view = src[:, 0:64][:, :, None].broadcast_to((128, N, N))
                    b_view = src[:, None, 64:128].broadcast_to((128, N, N))
                    nc.vector.scalar_tensor_tensor(
                        out=out3, in0=b_view, scalar=bias,
                        in1=a_view, op0=OP.add, op1=OP.add,
                    )
                    relu_eng = nc.gpsimd if GP_RELU else nc.vector
                    relu_eng.tensor_scalar_max(out=out3, in0=out3, scalar1=0.0)

                def build_edge(g):
                    e1 = e1pool.tile([128, N, N], f16, tag="e1", name=f"e1_{g}")
                    if g in PE_E_TILES:
                        build_pe(e1[:].rearrange("p i j -> p (i j)"), ab[g][:],
                                 be1_sb[:, 0:1], g in EVAC_DVE_E)
                    else:
                        build_dve(e1[:], ab[g][:], be1_sb[:, 0:1])
                    e1_tiles[g] = e1

                def build_strength(gr):
                    s1 = s1r[gr]
                    s3 = s1[:].rearrange("p (i j) -> p i j", i=N)
                    if gr in PE_S_TILES:
                        build_pe(s1[:], apbp[gr][:], bs1_sb[:, 0:1],
                                 gr in EVAC_DVE_S)
                    else:
                        build_dve(s3, apbp[gr][:], bs1_sb[:, 0:1])

                for u in range(4):
                    build_edge(2 * u)
                    build_edge(2 * u + 1)
                    build_strength(u)
                    e1a = e1_tiles[2 * u][:].rearrange("p i j -> p (i j)")
                    e1b = e1_tiles[2 * u + 1][:].rearrange("p i j -> p (i j)")
                    for cp in range(4):
                        ps = ps_e2.tile([128, 1024], f32, tag="e2", name="ps_e2_t")
                        for hf in range(2):
                            c0 = cp * 1024 + hf * 512
                            nc.tensor.matmul(
                                ps[0:64, hf * 512:(hf + 1) * 512],
                                lhsT=bd2we2_sb[:], rhs=e1a[:, c0:c0 + 512],
                                start=True, stop=True, tile_position=(0, 0),
                            )
                            nc.tensor.matmul(
                                ps[64:128, hf * 512:(hf + 1) * 512],
                                lhsT=bd2we2_sb[:], rhs=e1b[:, c0:c0 + 512],
                                start=True, stop=True, tile_position=(0, 64),
                            )
                        dst = e2r[u][:, cp * 1024:(cp + 1) * 1024]
                        if cp in E2RELU_DVE:
                            nc.vector.tensor_scalar(
                                out=dst, in0=ps[:], scalar1=be2_sb[:, 0:1],
                                scalar2=0.0, op0=OP.add, op1=OP.max)
                        else:
                            nc.scalar.activation(
                                out=dst, in_=ps[:], func=AF.Relu,
                                bias=be2_sb[:, 0:1], scale=1.0,
                            )
                    del e1_tiles[2 * u], e1_tiles[2 * u + 1]

            # ================= finals + sigmoid/tanh =================
            with tc.tile_pool(name="ps_fin", bufs=2, space="PSUM") as ps_fin, \
                 tc.tile_pool(name="outstage", bufs=2) as outstage:
                for half in range(2):
                    ps = ps_fin.tile([128, 2048], f32, tag="fin", name="ps_fin_t")
                    for u in range(4):
                        for ch in range(4):
                            c0 = half * 2048 + ch * 512
                            nc.tensor.matmul(
                                ps[32 * u:32 * u + 4, ch * 512:(ch + 1) * 512],
                                lhsT=bd4we3_sb[:], rhs=e2r[u][:, c0:c0 + 512],
                                start=True, stop=True, tile_position=(0, 32 * u),
                            )
                    adj_t = outstage.tile([128, 2048], f32, tag="adj", name="adj_t")
                    nc.scalar.activation(out=adj_t[:], in_=ps[:], func=AF.Sigmoid,
                                         bias=be3_sb[:, 0:1], scale=1.0)
                    for u in range(4):
                        nc.sync.dma_start(
                            out=adj_d[4 * u:4 * u + 4,
                                      half * 2048:(half + 1) * 2048],
                            in_=adj_t[32 * u:32 * u + 4, :])
                for half in range(2):
                    ps = ps_fin.tile([128, 2048], f32, tag="fin", name="ps_fin_t")
                    for gr in range(4):
                        for ch in range(4):
                            c0 = half * 2048 + ch * 512
                            nc.tensor.matmul(
                                ps[32 * gr:32 * gr + 4, ch * 512:(ch + 1) * 512],
                                lhsT=bd4ws2_sb[:], rhs=s1r[gr][:, c0:c0 + 512],
                                start=True, stop=True, tile_position=(0, 32 * gr),
                            )
                    str_t = outstage.tile([128, 2048], f32, tag="strt", name="str_t")
                    nc.scalar.activation(out=str_t[:], in_=ps[:], func=AF.Tanh,
                                         bias=bs2_sb[:, 0:1], scale=1.0)
                    for gr in range(4):
                        nc.sync.dma_start(
                            out=str_d[4 * gr:4 * gr + 4,
                                      half * 2048:(half + 1) * 2048],
                            in_=str_t[32 * gr:32 * gr + 4, :])

    _split_multiwait(nc)
    return nc


def _get_program():
    if "nc" not in _PROGRAM_CACHE:
        _PROGRAM_CACHE["nc"] = _build_program()
    return _PROGRAM_CACHE["nc"]


# ---------------------------------------------------------------------------
# host wrapper
# ---------------------------------------------------------------------------

def kernel(state_sequence, W1, b1, W2, b2, gamma, beta,
           We1, be1, We2, be2, We3, be3, Ws1, bs1, Ws2, bs2):
    global LAST_RESULTS
    from concourse.bass_utils import run_bass_kernel_spmd

    state_sequence = np.asarray(state_sequence, dtype=np.float32)
    W1 = np.asarray(W1, np.float32); b1 = np.asarray(b1, np.float32)
    W2 = np.asarray(W2, np.float32); b2 = np.asarray(b2, np.float32)
    gamma = np.asarray(gamma, np.float32); beta = np.asarray(beta, np.float32)
    We1 = np.asarray(We1, np.float32); be1 = np.asarray(be1, np.float32)
    We2 = np.asarray(We2, np.float32); be2 = np.asarray(be2, np.float32)
    We3 = np.asarray(We3, np.float32); be3 = np.asarray(be3, np.float32)
    Ws1 = np.asarray(Ws1, np.float32); bs1 = np.asarray(bs1, np.float32)
    Ws2 = np.asarray(Ws2, np.float32); bs2 = np.asarray(bs2, np.float32)

    idx = _node_indices()
    nodes = state_sequence[:, idx]                      # [B, N, D]

    def f16(a):
        return np.ascontiguousarray(a.astype(np.float16))

    def f32c(a):
        return np.ascontiguousarray(a.astype(np.float32))

    def blockdiag(w, k):
        m, n = w.shape
        out = np.zeros((m * k, n * k), np.float32)
        for i in range(k):
            out[i * m:(i + 1) * m, i * n:(i + 1) * n] = w
        return out

    parts16 = {
        "w1_0": W1[0:128], "w1_1": W1[128:256], "w1_2": W1[256:384],
        "w1_3": W1[384:512],
        "w2_0": W2[0:128], "w2_1": W2[128:256],
        "we1a": We1[:128, :], "we1b": We1[128:, :],
        "ws1a": Ws1[:128, :], "ws1b": Ws1[128:, :],
        "bd2we2": blockdiag(We2, 2),
        "bd4we3": blockdiag(We3, 4),
        "bd4ws2": blockdiag(Ws2, 4),
    }
    wb16 = np.zeros((128, W16_COLS), np.float16)
    for k, (o, w) in W16_OFF.items():
        wb16[:, o:o + w] = parts16[k].astype(np.float16)
    parts32 = {
        "b1_2": b1.reshape(2, 128).T, "b2c": b2.reshape(128, 1),
        "gammap": gamma.reshape(128, 1), "betap": beta.reshape(128, 1),
        "be1_2": np.tile(be1, 2).reshape(128, 1),
        "bs1_4": np.tile(bs1, 4).reshape(128, 1),
        "be2_4": np.tile(be2, 4).reshape(128, 1),
        "be3b": np.full((128, 1), float(be3[0])),
        "bs2b": np.full((128, 1), float(bs2[0])),
    }
    wb32 = np.zeros((128, W32_COLS), np.float32)
    for k, v in parts32.items():
        o = W32_OFF[k]
        wb32[:, o:o + v.shape[1]] = v
    shared = {"wb16": wb16, "wb32": f32c(wb32)}

    in_maps = []
    for c in range(NCORES):
        xt = f16(nodes[c * BSH:(c + 1) * BSH].reshape(TOK, D).T)
        in_maps.append({"xt": xt, **shared})

    nc = _get_program()
    res = run_bass_kernel_spmd(nc, in_maps, core_ids=list(range(NCORES)))
    LAST_RESULTS = res

    nf = np.empty((B, N, DM), np.float32)
    adj = np.empty((B, N, N), np.float32)
    stg = np.empty((B, N, N), np.float32)
    mask = (1.0 - np.eye(N, dtype=np.float32))
    for c in range(NCORES):
        r = res.results[c]
        nf_c = r["nf_raw"].reshape(BSH, N, DM) * gamma + beta
        nf[c * BSH:(c + 1) * BSH] = nf_c
        adj[c * BSH:(c + 1) * BSH] = r["adj"].reshape(BSH, N, N) * mask
        stg[c * BSH:(c + 1) * BSH] = r["strg"].reshape(BSH, N, N) * mask
    return nf, adj, stg


# revision 13
# speedup vs baseline: 3.3612x; 1.0925x over previous
"""CausalGraphBuilder Trainium2 kernel.

Full inputs -> shard batch (128) over 8 NeuronCores (16 each) -> Bass kernel
(encoder MLP + LayerNorm + N^2 pair-MLP edge/strength heads) -> gather.

Self-contained: hardcodes B,S,D,N = 128,1024,512,64 and the sharding.
"""

import numpy as np

B, S, D, N = 128, 1024, 512, 64
DM = D // 4          # 128 node-feature dim after encoder
NCORES = 8
BSH = B // NCORES    # 16 batch per core
TOK = BSH * N        # 1024 tokens per core

# jnp.linspace(0, S-1, N).astype(int32) as evaluated by the reference in this
# environment (device cast rounds); recomputed at runtime when jax is available.
_FALLBACK_IDX = [0, 16, 32, 49, 65, 81, 97, 114, 130, 146, 162, 179, 195, 211,
                 227, 244, 260, 276, 292, 309, 325, 341, 357, 373, 390, 406,
                 422, 438, 455, 471, 487, 503, 520, 536, 552, 568, 585, 601,
                 617, 633, 650, 666, 682, 698, 714, 731, 747, 763, 779, 796,
                 812, 828, 844, 861, 877, 893, 909, 926, 942, 958, 974, 991,
                 1007, 1023]


def _node_indices():
    try:
        import jax.numpy as jnp
        idx = np.asarray(jnp.linspace(0.0, float(S - 1), N).astype(jnp.int32))
        if idx.shape == (N,):
            return idx.astype(np.int64)
    except Exception:
        pass
    return np.array(_FALLBACK_IDX, dtype=np.int64)


# ---------------------------------------------------------------------------
# device program
# ---------------------------------------------------------------------------

_PROGRAM_CACHE = {}
LAST_RESULTS = None  # BassKernelResults of the most recent run (for test.py)

# engine assignment knobs (tuned against the profile)
PE_E_TILES = ()             # E-groups built on PE (identity-matmul 2-pass)
PE_S_TILES = (1, 3)     # S-groups built on PE
EVAC_DVE_E = ()               # PE-built E tiles evacuated by DVE instead of ACT
EVAC_DVE_S = ()
GP_RELU = False               # DVE-built tiles: relu on GPSIMD (slow ucode!)
E2RELU_DVE = ()               # e2 cp-chunks (0..3) whose relu-evac runs on DVE



# packed-constant blob layouts (columns)
_W16_SECTIONS = [
    ("w1_0", 256), ("w1_1", 256), ("w1_2", 256), ("w1_3", 256),
    ("w2_0", 128), ("w2_1", 128),
    ("we1a", 64), ("we1b", 64), ("ws1a", 32), ("ws1b", 32),
    ("bd2we2", 64), ("bd4we3", 4), ("bd4ws2", 4),
]
W16_OFF = {}
_o = 0
for _k, _w in _W16_SECTIONS:
    W16_OFF[_k] = (_o, _w)
    _o += _w
W16_COLS = _o
_W32_SECTIONS = [("b1_2", 2), ("b2c", 1), ("gammap", 1), ("betap", 1),
                 ("be1_2", 1), ("bs1_4", 1), ("be2_4", 1), ("be3b", 1),
                 ("bs2b", 1)]
W32_OFF = {}
_o = 0
for _k, _w in _W32_SECTIONS:
    W32_OFF[_k] = _o
    _o += _w
W32_COLS = _o

def _patched_tile_context(nc):
    """TileContext whose tail drain never carries >1 sync wait (this walrus
    build rejects multi-wait CTRL instructions)."""
    import concourse.mybir as mybir
    import concourse.tile as tile
    from concourse.vector_clock import ScopedClock

    class TileContextP(tile.TileContext):
        def _drain_and_barrier(self, tick_clock, wait_clock):
            drain_inst = self.nc.sync.drain()
            wait_clock.add_sem_waits(
                drain_inst.ins, ScopedClock({None: tick_clock.global_clock})
            )
            si = drain_inst.ins.sync_info
            if si is not None and si.on_wait is not None and len(si.on_wait) > 1:
                waits = list(si.on_wait)
                si.on_wait = waits[:1]
                for w in waits[1:]:
                    extra = self.nc.sync.drain()
                    extra.ins.sync_info = mybir.SyncInfo(on_wait=[w], on_update=[])
            self.nc.all_engine_barrier()
            popped = self.nc._tile_sem_poison_stack.pop()
            assert popped is self._sem_poison
            self.nc.clear_and_free_semaphores(list(self.sems.allocated().values()))

    return TileContextP(nc)


def _split_multiwait(nc):
    """This walrus build accepts at most ONE sync wait per instruction; hoist
    extra waits into single-wait NoOps on the same engine just before."""
    import concourse.mybir as mybir

    n_split = 0
    for f in nc.m.functions:
        for bb in f.blocks:
            insts = list(bb.instructions)
            out = []
            for ins in insts:
                si = ins.sync_info
                if si is not None and si.on_wait is not None and len(si.on_wait) > 1:
                    waits = list(si.on_wait)
                    for w in waits[:-1]:
                        nop = mybir.InstNoOp(
                            name=f"{ins.name}-w{n_split}",
                            engine=ins.engine,
                            bass_nofuse=True,
                            sync_info=mybir.SyncInfo(on_wait=[w], on_update=[]),
                        )
                        out.append(nop)
                        n_split += 1
                    si.on_wait = waits[-1:]
                out.append(ins)
            if n_split:
                bb.instructions = out
    return n_split


def _build_program():
    import concourse.bass as bass
    import concourse.mybir as mybir
    from concourse.masks import make_identity

    f16 = mybir.dt.float16
    f32 = mybir.dt.float32
    AF = mybir.ActivationFunctionType
    OP = mybir.AluOpType

    nc = bass.Bass()

    # ---- DRAM I/O ----
    xt_d = nc.dram_tensor("xt", [D, TOK], f16, kind="ExternalInput")
    wb16_d = nc.dram_tensor("wb16", [128, W16_COLS], f16, kind="ExternalInput")
    wb32_d = nc.dram_tensor("wb32", [128, W32_COLS], f32, kind="ExternalInput")

    nf_d = nc.dram_tensor("nf_raw", [TOK, DM], f32, kind="ExternalOutput")
    adj_d = nc.dram_tensor("adj", [BSH, N * N], f32, kind="ExternalOutput")
    str_d = nc.dram_tensor("strg", [BSH, N * N], f32, kind="ExternalOutput")

    with _patched_tile_context(nc) as tc:
        from contextlib import ExitStack

        with ExitStack() as ctx:
            consts = ctx.enter_context(tc.tile_pool(name="consts", bufs=1))
            persist = ctx.enter_context(tc.tile_pool(name="persist", bufs=1))

            # ---- load constants ----
            def load(pool, name, dram, shape, dtype):
                t = pool.tile(shape, dtype, tag=name, name=name)
                nc.sync.dma_start(out=t[:], in_=dram[:])
                return t

            xt_sb = [load(consts, f"xt{c}", xt_d[c * 128:(c + 1) * 128, :],
                          [128, TOK], f16) for c in range(4)]
            wb16 = load(consts, "wb16", wb16_d, [128, W16_COLS], f16)
            wb32 = load(consts, "wb32", wb32_d, [128, W32_COLS], f32)

            def c16(key):
                o, w = W16_OFF[key]
                return wb16[:, o:o + w]

            def c32(key):
                o = W32_OFF[key]
                return wb32[:, o:o + 1]

            w1_sb = [c16(f"w1_{c}") for c in range(4)]
            w2_sb = [c16(f"w2_{c}") for c in range(2)]
            we1a_sb = c16("we1a"); we1b_sb = c16("we1b")
            ws1a_sb = c16("ws1a"); ws1b_sb = c16("ws1b")
            bd2we2_sb = c16("bd2we2")
            bd4we3_sb = c16("bd4we3"); bd4ws2_sb = c16("bd4ws2")
            b1_sb = wb32[:, W32_OFF["b1_2"]:W32_OFF["b1_2"] + 2]
            b2_sb = c32("b2c"); gp_sb = c32("gammap"); bp_sb = c32("betap")
            be1_sb = c32("be1_2"); bs1_sb = c32("bs1_4"); be2_sb = c32("be2_4")
            be3_sb = c32("be3b"); bs2_sb = c32("bs2b")

            eps_sb = consts.tile([128, 1], f32, tag="eps")
            nc.vector.memset(eps_sb[:], 1e-5)

            idf32 = consts.tile([128, 128], f32, tag="idf32")
            make_identity(nc, idf32[:])
            idf16 = consts.tile([128, 128], f16, tag="idf16")
            make_identity(nc, idf16[:])

            h1t = [persist.tile([128, TOK], f16, tag=f"h1t{m}", name=f"h1t{m}") for m in range(2)]
            ht = persist.tile([128, TOK], f32, tag="ht")
            nfraw = persist.tile([128, 8, 128], f32, tag="nfraw")
            nft = persist.tile([128, TOK], f16, tag="nft")
            ab = [persist.tile([128, 192], f16, tag=f"ab{g}", name=f"ab{g}") for g in range(8)]
            apbp = [persist.tile([128, 192], f16, tag=f"apbp{gr}", name=f"apbp{gr}") for gr in range(4)]
            e2r = [persist.tile([128, N * N], f16, tag=f"e2r{u}", name=f"e2r{u}") for u in range(4)]
            s1r = [persist.tile([128, N * N], f16, tag=f"s1r{gr}", name=f"s1r{gr}") for gr in range(4)]

            # ================= encoder =================
            with tc.tile_pool(name="ps_enc", bufs=2, space="PSUM") as ps_enc:
                for m in range(2):
                    for n2 in range(2):
                        ps = ps_enc.tile([128, 512], f32, tag="enc", name="ps_enc_t")
                        for c in range(4):
                            nc.tensor.matmul(
                                ps[:],
                                lhsT=w1_sb[c][:, m * 128:(m + 1) * 128],
                                rhs=xt_sb[c][:, n2 * 512:(n2 + 1) * 512],
                                start=(c == 0), stop=(c == 3),
                            )
                        nc.scalar.activation(
                            out=h1t[m][:, n2 * 512:(n2 + 1) * 512], in_=ps[:],
                            func=AF.Relu, bias=b1_sb[:, m:m + 1], scale=1.0,
                        )
                for n2 in range(2):
                    ps = ps_enc.tile([128, 512], f32, tag="enc", name="ps_enc_t")
                    for c2 in range(2):
                        nc.tensor.matmul(
                            ps[:], lhsT=w2_sb[c2][:],
                            rhs=h1t[c2][:, n2 * 512:(n2 + 1) * 512],
                            start=(c2 == 0), stop=(c2 == 1),
                        )
                    nc.scalar.activation(
                        out=ht[:, n2 * 512:(n2 + 1) * 512], in_=ps[:],
                        func=AF.Identity, bias=b2_sb[:, 0:1], scale=1.0,
                    )

            # ================= layernorm (token-major) =================
            with tc.tile_pool(name="ps_tr", bufs=3, space="PSUM") as ps_tr, \
                 tc.tile_pool(name="ln_tmp", bufs=4) as ln_tmp:
                for t in range(8):
                    pst = ps_tr.tile([128, 128], f32, tag="htok", name="pst")
                    nc.tensor.transpose(pst[:], ht[:, t * 128:(t + 1) * 128],
                                        idf32[:])
                    st6 = ln_tmp.tile([128, 6], f32, tag="st6", name="st6")
                    nc.vector.bn_stats(out=st6[:], in_=pst[:])
                    mv = ln_tmp.tile([128, 2], f32, tag="mv", name="mv")
                    nc.vector.bn_aggr(out=mv[:], in_=st6[:])
                    rstd = ln_tmp.tile([128, 1], f32, tag="rstd", name="rstd")
                    nc.scalar.activation(out=rstd[:], in_=mv[:, 1:2],
                                         func=AF.Sqrt, bias=eps_sb[:, 0:1],
                                         scale=1.0)
                    nc.vector.reciprocal(out=rstd[:], in_=rstd[:])
                    nc.vector.tensor_scalar(
                        out=nfraw[:, t, :], in0=pst[:],
                        scalar1=mv[:, 0:1], scalar2=rstd[:, 0:1],
                        op0=OP.subtract, op1=OP.mult,
                    )
                    nc.sync.dma_start(out=nf_d[t * 128:(t + 1) * 128, :],
                                      in_=nfraw[:, t, :])
                # nfT (feature-major) with gamma/beta applied per-partition
                for t in range(8):
                    psn = ps_tr.tile([128, 128], f32, tag="nft_ps", name="psn")
                    nc.tensor.transpose(psn[:], nfraw[:, t, :], idf32[:])
                    nc.vector.tensor_scalar(
                        out=nft[:, t * 128:(t + 1) * 128], in0=psn[:],
                        scalar1=gp_sb[:, 0:1], scalar2=bp_sb[:, 0:1],
                        op0=OP.mult, op1=OP.add,
                    )

            # ================= pair projections =================
            with tc.tile_pool(name="ps_proj", bufs=3, space="PSUM") as ps_proj:
                for g in range(8):           # edge groups: b = 2g, 2g+1
                    ps = ps_proj.tile([128, 192], f32, tag="proj_e", name="ps_proj_e")
                    for bb in range(2):
                        b = 2 * g + bb
                        cols = nft[:, b * N:(b + 1) * N]
                        cols_dup = cols[:, :, None].broadcast_to((128, N, 2))
                        nc.tensor.matmul(
                            ps[bb * 64:(bb + 1) * 64, 0:64], lhsT=we1a_sb[:],
                            rhs=cols, start=True, stop=True,
                            tile_position=(0, 64 * bb),
                        )
                        nc.tensor.matmul(
                            ps[bb * 64:(bb + 1) * 64, 64:192], lhsT=we1b_sb[:],
                            rhs=cols_dup, start=True, stop=True,
                            tile_position=(0, 64 * bb),
                        )
                    nc.scalar.activation(out=ab[g][:, 0:64], in_=ps[:, 0:64],
                                         func=AF.Identity, bias=be1_sb[:, 0:1],
                                         scale=1.0)
                    nc.scalar.activation(out=ab[g][:, 64:192], in_=ps[:, 64:192],
                                         func=AF.Copy)
                for gr in range(4):          # strength groups: b = 4gr..4gr+3
                    ps = ps_proj.tile([128, 192], f32, tag="proj_s", name="ps_proj_s")
                    for bb in range(4):
                        b = 4 * gr + bb
                        cols = nft[:, b * N:(b + 1) * N]
                        cols_dup = cols[:, :, None].broadcast_to((128, N, 2))
                        nc.tensor.matmul(
                            ps[bb * 32:(bb + 1) * 32, 0:64], lhsT=ws1a_sb[:],
                            rhs=cols, start=True, stop=True,
                            tile_position=(0, 32 * bb),
                        )
                        nc.tensor.matmul(
                            ps[bb * 32:(bb + 1) * 32, 64:192], lhsT=ws1b_sb[:],
                            rhs=cols_dup, start=True, stop=True,
                            tile_position=(0, 32 * bb),
                        )
                    nc.scalar.activation(out=apbp[gr][:, 0:64], in_=ps[:, 0:64],
                                         func=AF.Identity, bias=bs1_sb[:, 0:1],
                                         scale=1.0)
                    nc.scalar.activation(out=apbp[gr][:, 64:192],
                                         in_=ps[:, 64:192], func=AF.Copy)

            # ================= N^2 builds + edge layer2 =================
            with tc.tile_pool(name="e1pool", bufs=3) as e1pool, \
                 tc.tile_pool(name="ps_bld", bufs=2, space="PSUM") as ps_bld, \
                 tc.tile_pool(name="ps_e2", bufs=2, space="PSUM") as ps_e2:
                e1_tiles = {}

                def build_pe(out_flat, src, bias, evac_dve):
                    """2-pass identity-matmul broadcast build + relu evac.

                    src: [128, 192] sbuf tile, A in cols 0:64 (i-side),
                    B duplicated in cols 64:192 (j-side, each col twice).
                    Pair order: col = j*64 + i."""
                    bdup3 = src[:, 64:192].rearrange("p (j ii) -> p j ii", ii=2)
                    for q in range(4):           # 1024-pair chunks (16 j vals)
                        ps = ps_bld.tile([128, 1024], f32, tag="bld",
                                         name="ps_bld_t")
                        for hf in range(2):      # 512-col matmuls (8 j vals)
                            j0 = q * 16 + hf * 8
                            a_chunk = src[:, None, 0:64] \
                                .broadcast_to((128, 8, 64))
                            b_chunk = bdup3[:, j0:j0 + 8, 0:1] \
                                .broadcast_to((128, 8, 64))
                            o = ps[:, hf * 512:(hf + 1) * 512]
                            nc.tensor.matmul(o, lhsT=idf16[:], rhs=a_chunk,
                                             start=True, stop=False)
                            nc.tensor.matmul(o, lhsT=idf16[:], rhs=b_chunk,
                                             start=False, stop=True)
                        dst = out_flat[:, q * 1024:(q + 1) * 1024]
                        if evac_dve:
                            nc.vector.tensor_scalar_max(
                                out=dst, in0=ps[:], scalar1=0.0)
                        else:
                            nc.scalar.activation(out=dst, in_=ps[:],
                                                 func=AF.Relu, scale=1.0)

                def build_dve(out3, src, bias):
                    # 4D views, all innermost step 1 (2x-mode eligible):
                    # out[p, j, i2, ii], i = 2*i2 + ii
                    out4 = out3.rearrange("p j (i2 ii) -> p j i2 ii", ii=2)
                    a4 = src[:, None, 0:64].broadcast_to((128, N, 64)) \
                        .rearrange("p j (i2 ii) -> p j i2 ii", ii=2)
                    b4 = src[:, 64:192].rearrange("p (j ii) -> p j ii", ii=2)[
                        :, :, None, :].broadcast_to((128, N, 32, 2))
                    nc.vector.tensor_add(out4, b4, a4)
                    relu_eng = nc.gpsimd if GP_RELU else nc.vector
                    relu_eng.tensor_scalar_max(out=out3, in0=out3, scalar1=0.0)

                def build_edge(g):
                    e1 = e1pool.tile([128, N, N], f16, tag="e1", name=f"e1_{g}")
                    if g in PE_E_TILES:
                        build_pe(e1[:].rearrange("p i j -> p (i j)"), ab[g][:],
                                 be1_sb[:, 0:1], g in EVAC_DVE_E)
                    else:
                        build_dve(e1[:], ab[g][:], be1_sb[:, 0:1])
                    e1_tiles[g] = e1

                def build_strength(gr):
                    s1 = s1r[gr]
                    s3 = s1[:].rearrange("p (i j) -> p i j", i=N)
                    if gr in PE_S_TILES:
                        build_pe(s1[:], apbp[gr][:], bs1_sb[:, 0:1],
                                 gr in EVAC_DVE_S)
                    else:
                        build_dve(s3, apbp[gr][:], bs1_sb[:, 0:1])

                for u in range(4):
                    build_edge(2 * u)
                    build_edge(2 * u + 1)
                    build_strength(u)
                    e1a = e1_tiles[2 * u][:].rearrange("p i j -> p (i j)")
                    e1b = e1_tiles[2 * u + 1][:].rearrange("p i j -> p (i j)")
                    for cp in range(4):
                        ps = ps_e2.tile([128, 1024], f32, tag="e2", name="ps_e2_t")
                        for hf in range(2):
                            c0 = cp * 1024 + hf * 512
                            nc.tensor.matmul(
                                ps[0:64, hf * 512:(hf + 1) * 512],
                                lhsT=bd2we2_sb[:], rhs=e1a[:, c0:c0 + 512],
                                start=True, stop=True, tile_position=(0, 0),
                            )
                            nc.tensor.matmul(
                                ps[64:128, hf * 512:(hf + 1) * 512],
                                lhsT=bd2we2_sb[:], rhs=e1b[:, c0:c0 + 512],
                                start=True, stop=True, tile_position=(0, 64),
                            )
                        dst = e2r[u][:, cp * 1024:(cp + 1) * 1024]
                        if cp in E2RELU_DVE:
                            nc.vector.tensor_scalar(
                                out=dst, in0=ps[:], scalar1=be2_sb[:, 0:1],
                                scalar2=0.0, op0=OP.add, op1=OP.max)
                        else:
                            nc.scalar.activation(
                                out=dst, in_=ps[:], func=AF.Relu,
                                bias=be2_sb[:, 0:1], scale=1.0,
                            )
                    del e1_tiles[2 * u], e1_tiles[2 * u + 1]

            # ================= finals + sigmoid/tanh =================
            with tc.tile_pool(name="ps_fin", bufs=2, space="PSUM") as ps_fin, \
                 tc.tile_pool(name="outstage", bufs=2) as outstage:
                for half in range(2):
                    ps = ps_fin.tile([128, 2048], f32, tag="fin", name="ps_fin_t")
                    for u in range(4):
                        for ch in range(4):
                            c0 = half * 2048 + ch * 512
                            nc.tensor.matmul(
                                ps[32 * u:32 * u + 4, ch * 512:(ch + 1) * 512],
                                lhsT=bd4we3_sb[:], rhs=e2r[u][:, c0:c0 + 512],
                                start=True, stop=True, tile_position=(0, 32 * u),
                            )
                    adj_t = outstage.tile([128, 2048], f32, tag="adj", name="adj_t")
                    nc.scalar.activation(out=adj_t[:], in_=ps[:], func=AF.Sigmoid,
                                         bias=be3_sb[:, 0:1], scale=1.0)
                    for u in range(4):
                        nc.sync.dma_start(
                            out=adj_d[4 * u:4 * u + 4,
                                      half * 2048:(half + 1) * 2048],
                            in_=adj_t[32 * u:32 * u + 4, :])
                for half in range(2):
                    ps = ps_fin.tile([128, 2048], f32, tag="fin", name="ps_fin_t")
                    for gr in range(4):
                        for ch in range(4):
                            c0 = half * 2048 + ch * 512
                            nc.tensor.matmul(
                                ps[32 * gr:32 * gr + 4, ch * 512:(ch + 1) * 512],
                                lhsT=bd4ws2_sb[:], rhs=s1r[gr][:, c0:c0 + 512],
                                start=True, stop=True, tile_position=(0, 32 * gr),
                            )
                    str_t = outstage.tile([128, 2048], f32, tag="strt", name="str_t")
                    nc.scalar.activation(out=str_t[:], in_=ps[:], func=AF.Tanh,
                                         bias=bs2_sb[:, 0:1], scale=1.0)
                    for gr in range(4):
                        nc.sync.dma_start(
                            out=str_d[4 * gr:4 * gr + 4,
                                      half * 2048:(half + 1) * 2048],
                            in_=str_t[32 * gr:32 * gr + 4, :])

    _split_multiwait(nc)
    return nc


def _get_program():
    if "nc" not in _PROGRAM_CACHE:
        _PROGRAM_CACHE["nc"] = _build_program()
    return _PROGRAM_CACHE["nc"]


# ---------------------------------------------------------------------------
# host wrapper
# ---------------------------------------------------------------------------

def kernel(state_sequence, W1, b1, W2, b2, gamma, beta,
           We1, be1, We2, be2, We3, be3, Ws1, bs1, Ws2, bs2):
    global LAST_RESULTS
    from concourse.bass_utils import run_bass_kernel_spmd

    state_sequence = np.asarray(state_sequence, dtype=np.float32)
    W1 = np.asarray(W1, np.float32); b1 = np.asarray(b1, np.float32)
    W2 = np.asarray(W2, np.float32); b2 = np.asarray(b2, np.float32)
    gamma = np.asarray(gamma, np.float32); beta = np.asarray(beta, np.float32)
    We1 = np.asarray(We1, np.float32); be1 = np.asarray(be1, np.float32)
    We2 = np.asarray(We2, np.float32); be2 = np.asarray(be2, np.float32)
    We3 = np.asarray(We3, np.float32); be3 = np.asarray(be3, np.float32)
    Ws1 = np.asarray(Ws1, np.float32); bs1 = np.asarray(bs1, np.float32)
    Ws2 = np.asarray(Ws2, np.float32); bs2 = np.asarray(bs2, np.float32)

    idx = _node_indices()
    nodes = state_sequence[:, idx]                      # [B, N, D]

    def f16(a):
        return np.ascontiguousarray(a.astype(np.float16))

    def f32c(a):
        return np.ascontiguousarray(a.astype(np.float32))

    def blockdiag(w, k):
        m, n = w.shape
        out = np.zeros((m * k, n * k), np.float32)
        for i in range(k):
            out[i * m:(i + 1) * m, i * n:(i + 1) * n] = w
        return out

    parts16 = {
        "w1_0": W1[0:128], "w1_1": W1[128:256], "w1_2": W1[256:384],
        "w1_3": W1[384:512],
        "w2_0": W2[0:128], "w2_1": W2[128:256],
        "we1a": We1[:128, :], "we1b": We1[128:, :],
        "ws1a": Ws1[:128, :], "ws1b": Ws1[128:, :],
        "bd2we2": blockdiag(We2, 2),
        "bd4we3": blockdiag(We3, 4),
        "bd4ws2": blockdiag(Ws2, 4),
    }
    wb16 = np.zeros((128, W16_COLS), np.float16)
    for k, (o, w) in W16_OFF.items():
        wb16[:, o:o + w] = parts16[k].astype(np.float16)
    parts32 = {
        "b1_2": b1.reshape(2, 128).T, "b2c": b2.reshape(128, 1),
        "gammap": gamma.reshape(128, 1), "betap": beta.reshape(128, 1),
        "be1_2": np.tile(be1, 2).reshape(128, 1),
        "bs1_4": np.tile(bs1, 4).reshape(128, 1),
        "be2_4": np.tile(be2, 4).reshape(128, 1),
        "be3b": np.full((128, 1), float(be3[0])),
        "bs2b": np.full((128, 1), float(bs2[0])),
    }
    wb32 = np.zeros((128, W32_COLS), np.float32)
    for k, v in parts32.items():
        o = W32_OFF[k]
        wb32[:, o:o + v.shape[1]] = v
    shared = {"wb16": wb16, "wb32": f32c(wb32)}

    in_maps = []
    for c in range(NCORES):
        xt = f16(nodes[c * BSH:(c + 1) * BSH].reshape(TOK, D).T)
        in_maps.append({"xt": xt, **shared})

    nc = _get_program()
    res = run_bass_kernel_spmd(nc, in_maps, core_ids=list(range(NCORES)))
    LAST_RESULTS = res

    nf = np.empty((B, N, DM), np.float32)
    adj = np.empty((B, N, N), np.float32)
    stg = np.empty((B, N, N), np.float32)
    mask = (1.0 - np.eye(N, dtype=np.float32))
    for c in range(NCORES):
        r = res.results[c]
        nf_c = r["nf_raw"].reshape(BSH, N, DM) * gamma + beta
        nf[c * BSH:(c + 1) * BSH] = nf_c
        adj[c * BSH:(c + 1) * BSH] = \
            r["adj"].reshape(BSH, N, N).transpose(0, 2, 1) * mask
        stg[c * BSH:(c + 1) * BSH] = \
            r["strg"].reshape(BSH, N, N).transpose(0, 2, 1) * mask
    return nf, adj, stg
